# revision 1
# baseline (speedup 1.0000x reference)
"""Trainium2 Bass kernel for nn_Decomposeable (decomposable attention model).

Strategy: data-parallel over batch B=128 across 8 NeuronCores (16 items/core).
Embedding table replicated; rows gathered on-device via indirect DMA and
L2-normalized on-device. All matmuls run as float32r (FP22 multiply, fp32
accumulate) at full PE rate; feature-major layouts keep every matmul's
moving free dim at 256 columns.
"""
import sys
import numpy as np

for _p in ("/opt/trn_rl_repo",):
    if _p not in sys.path:
        sys.path.append(_p)

import concourse.bass as bass
import concourse.bacc as bacc
import concourse.tile as tile
from concourse import mybir
from concourse.bass_utils import run_bass_kernel_spmd
from concourse.masks import make_identity

F32 = mybir.dt.float32
F32R = mybir.dt.float32r
I32 = mybir.dt.int32
AF = mybir.ActivationFunctionType
ALU = mybir.AluOpType
AX = mybir.AxisListType

L, EMB, PROJ, ATT, CLS = 256, 300, 200, 200, 3
B, NCORES = 128, 8
NIT = B // NCORES            # items per core
VOCAB = 50000

D_SL = [(0, 128), (128, 256), (256, 300)]
T_SL = [(0, 128), (128, 256)]
P_SL = [(0, 128), (128, 200)]
V_SL = [(0, 128), (128, 256), (256, 384), (384, 400)]
WC_K = [(0, 128), (128, 200), (200, 328), (328, 400)]   # rows of Wc per K-tile
WG_K = [(s + v0, s + v1) for s in (0, 400) for (v0, v1) in V_SL]

_CACHED_NC = None


def _build_nc():
    nc = bacc.Bacc("TRN2", target_bir_lowering=False, debug=False)

    dram = {}
    def din(name, shape, dt=F32):
        dram[name] = nc.dram_tensor(name, shape, dt, kind="ExternalInput")
        return dram[name]

    din("idx1", [128, 2 * NIT], I32)
    din("idx2", [128, 2 * NIT], I32)
    din("xi1", [NIT, L], I32)
    din("xi2", [NIT, L], I32)
    din("emb", [VOCAB, EMB])
    din("wi", [EMB, ATT]); din("bi", [ATT, 1])
    din("wp", [2 * EMB, PROJ]); din("bp", [PROJ, 1])
    din("wa", [PROJ, ATT]); din("ba", [ATT, 1])
    din("wc", [2 * PROJ, 2 * PROJ]); din("bc", [2 * PROJ, 1])
    din("wg", [4 * PROJ, CLS]); din("bg", [CLS, 1])
    din("bdist", [128, 1])
    din("dmask", [L, L])
    out_d = nc.dram_tensor("out", [CLS, NIT], F32, kind="ExternalOutput")

    with tile.TileContext(nc) as tc:
        _emit(nc, tc, dram, out_d)
    nc.compile()
    return nc


def _emit(nc, tc, dram, out_d):
    from contextlib import ExitStack
    ctx = ExitStack()
    with ctx:
        C = ctx.enter_context(tc.tile_pool(name="consts", bufs=1))
        PS256 = ctx.enter_context(tc.tile_pool(name="ps256", bufs=3, space="PSUM"))
        PST = ctx.enter_context(tc.tile_pool(name="psT", bufs=3, space="PSUM"))
        PSS = ctx.enter_context(tc.tile_pool(name="pss", bufs=1, space="PSUM"))
        PSA = ctx.enter_context(tc.tile_pool(name="psagg", bufs=1, space="PSUM"))
        W6 = ctx.enter_context(tc.tile_pool(name="work6", bufs=6))
        W8 = ctx.enter_context(tc.tile_pool(name="work8", bufs=8))
        W4 = ctx.enter_context(tc.tile_pool(name="work4", bufs=4))
        W10 = ctx.enter_context(tc.tile_pool(name="work10", bufs=10))
        SCR = ctx.enter_context(tc.tile_pool(name="scratch", bufs=3))

        # ---- constants
        ident = C.tile([128, 128], F32)
        make_identity(nc, ident[:])
        identr = C.tile([128, 128], F32R)
        nc.vector.tensor_copy(identr[:], ident[:])
        ones_f = C.tile([1, 128], F32)
        nc.vector.memset(ones_f[:], 1.0)
        ones_r = C.tile([1, 128], F32R)
        nc.vector.tensor_copy(ones_r[:], ones_f[:])
        iota_i = C.tile([128, L], I32)
        nc.gpsimd.iota(iota_i[:], pattern=[[1, L]], base=0, channel_multiplier=0)
        iotaB = C.tile([128, L], F32)
        nc.vector.tensor_copy(iotaB[:], iota_i[:])

        def load(name, r0, r1, dt=F32R, cols=None):
            src = dram[name].ap()
            if cols is not None:
                src = src[:, cols[0]:cols[1]]
            w = src.shape[1]
            t = C.tile([128, w], dt, tag=f"{name}_{r0}")
            nc.sync.dma_start(out=t[:r1 - r0, :], in_=src[r0:r1, :].bitcast(dt))
            return t

        wi_k = [load("wi", d0, d1) for (d0, d1) in D_SL]
        wp_k = [load("wp", d0, d1) for (d0, d1) in D_SL] + \
               [load("wp", 300 + d0, 300 + d1) for (d0, d1) in D_SL]
        wa_k = [load("wa", p0, p1) for (p0, p1) in P_SL]
        wc_k = [load("wc", k0, k1) for (k0, k1) in WC_K]
        wg_k = [load("wg", k0, k1) for (k0, k1) in WG_K]
        bi_t = [load("bi", a0, a1, F32) for (a0, a1) in P_SL]
        bp_t = [load("bp", p0, p1, F32) for (p0, p1) in P_SL]
        ba_t = [load("ba", a0, a1, F32) for (a0, a1) in P_SL]
        bc_t = [load("bc", v0, v1, F32) for (v0, v1) in V_SL]
        bg_t = load("bg", 0, CLS, F32)
        bdist = load("bdist", 0, 128, F32)
        dmask_t = [load("dmask", t0, t1, F32) for (t0, t1) in T_SL]
        bias2d = []
        for mi in range(2):
            b2 = C.tile([128, L], F32, tag=f"bias2d_{mi}")
            nc.vector.tensor_scalar_mul(b2[:], dmask_t[mi][:], bdist[:, :1])
            bias2d.append(b2)

        idx_sb = {}
        for s, name in ((1, "idx1"), (2, "idx2")):
            t = C.tile([128, 2 * NIT], I32, tag=name)
            nc.sync.dma_start(out=t[:], in_=dram[name].ap())
            idx_sb[s] = t

        # ---- per-side masks: m_all -> mcol tiles + sizebc
        mcol = {}
        sizebc = {}
        m_all_t = {}
        for s, name in ((1, "xi1"), (2, "xi2")):
            xi = C.tile([NIT, L], I32, tag=name)
            nc.sync.dma_start(out=xi[:], in_=dram[name].ap())
            xf = SCR.tile([NIT, L], F32, tag="xf")
            nc.vector.tensor_copy(xf[:], xi[:])
            nz = SCR.tile([NIT, L], F32, tag="nz")
            nc.vector.tensor_scalar(nz[:], xf[:], 0.0, None, op0=ALU.not_equal)
            sizes = C.tile([NIT, 1], F32, tag=f"sizes{s}")
            nc.vector.tensor_reduce(sizes[:], nz[:], axis=AX.X, op=ALU.add)
            m_all = C.tile([NIT, L], F32, tag=f"mall{s}")
            nc.vector.tensor_scalar(m_all[:], iotaB[:NIT, :], sizes[:, :1], None,
                                    op0=ALU.is_lt)
            m_all_t[s] = m_all
            cols = []
            for ti, (t0, t1) in enumerate(T_SL):
                pt = PST.tile([128, 128], F32, tag="psT")
                nc.tensor.transpose(pt[:, :NIT], m_all[:, t0:t1], ident[:NIT, :NIT])
                mc = C.tile([128, NIT], F32, tag=f"mcol{s}_{ti}")
                nc.vector.tensor_copy(mc[:], pt[:, :NIT])
                cols.append(mc)
            mcol[s] = cols
            # sizes row -> broadcast down partitions
            pt = PSS.tile([128, NIT], F32, tag="pss")
            nc.tensor.transpose(pt[:1, :NIT], sizes[:, :1], ident[:NIT, :NIT])
            srow = C.tile([1, NIT], F32R, tag=f"srow{s}")
            nc.vector.tensor_copy(srow[:], pt[:1, :NIT])
            pb = PSS.tile([128, NIT], F32, tag="pss")
            nc.tensor.matmul(pb[:, :], lhsT=ones_r[:], rhs=srow[:], start=True, stop=True)
            sb = C.tile([128, NIT], F32, tag=f"sizebc{s}")
            nc.vector.tensor_copy(sb[:], pb[:])
            sizebc[s] = sb

        # pooled accumulators [vsz, NIT] per (side, vtile)
        pooled = {(s, vi): C.tile([128, NIT], F32, tag=f"pool{s}_{vi}",
                                  name=f"pool{s}_{vi}")
                  for s in (1, 2) for vi in range(4)}

        # ================= per-item pipeline =================
        def intra(side, it):
            """Embed + intra-attention + projection for one item side.
            Returns (e_n tiles, pT tiles, pRow tiles, aT tiles)."""
            e_n = []
            for ti in range(2):
                g = 2 * it + ti
                e_raw = W4.tile([128, EMB], F32, tag="eraw")
                nc.gpsimd.indirect_dma_start(
                    out=e_raw[:], out_offset=None, in_=dram["emb"].ap(),
                    in_offset=bass.IndirectOffsetOnAxis(
                        ap=idx_sb[side][:, g:g + 1], axis=0))
                sq = SCR.tile([128, EMB], F32, tag="sq")
                ss = SCR.tile([128, 1], F32, tag="ss")
                nc.scalar.activation(sq[:], e_raw[:], AF.Square, accum_out=ss[:, :1])
                lns = SCR.tile([128, 1], F32, tag="lns")
                nc.scalar.activation(lns[:], ss[:], AF.Ln)
                inv = SCR.tile([128, 1], F32, tag="inv")
                nc.scalar.activation(inv[:], lns[:], AF.Exp, scale=-0.5)
                en = W8.tile([128, EMB], F32R, tag="en")
                nc.vector.tensor_scalar_mul(en[:], e_raw[:], inv[:, :1])
                e_n.append(en)

            # eT[di]: [dsz, 256]
            eT = []
            for di, (d0, d1) in enumerate(D_SL):
                dsz = d1 - d0
                t = W8.tile([128, L], F32R, tag="eT")
                for ti, (t0, t1) in enumerate(T_SL):
                    pt = PST.tile([128, 128], F32R, tag="psT")
                    nc.tensor.transpose(pt[:dsz, :], e_n[ti][:, d0:d1], identr[:])
                    nc.vector.tensor_copy(t[:dsz, t0:t1], pt[:dsz, :])
                eT.append(t)

            # fT = relu(Wi^T eT + bi)
            fT = []
            for ai, (a0, a1) in enumerate(P_SL):
                asz = a1 - a0
                ps = PS256.tile([128, L], F32, tag="ps256")
                for k in range(3):
                    ksz = D_SL[k][1] - D_SL[k][0]
                    nc.tensor.matmul(ps[:asz, :], lhsT=wi_k[k][:ksz, a0:a1],
                                     rhs=eT[k][:ksz, :], start=(k == 0), stop=(k == 2))
                t = W6.tile([128, L], F32R, tag="fT")
                nc.scalar.activation(t[:asz, :], ps[:asz, :], AF.Relu,
                                     bias=bi_t[ai][:asz, :1])
                fT.append(t)

            # att = fT^T fT + bias2d ; softmax rows -> S
            S = []
            for mi, (m0, m1) in enumerate(T_SL):
                ps = PS256.tile([128, L], F32, tag="ps256")
                for ai, (a0, a1) in enumerate(P_SL):
                    asz = a1 - a0
                    nc.tensor.matmul(ps[:, :], lhsT=fT[ai][:asz, m0:m1],
                                     rhs=fT[ai][:asz, :],
                                     start=(ai == 0), stop=(ai == 1))
                att = W6.tile([128, L], F32, tag="att")
                nc.vector.tensor_add(att[:], ps[:], bias2d[mi][:])
                mx = SCR.tile([128, 1], F32, tag="mx")
                nc.vector.tensor_reduce(mx[:], att[:], axis=AX.X, op=ALU.max,
                                        negate=True)
                den = SCR.tile([128, 1], F32, tag="den")
                nc.scalar.activation(att[:], att[:], AF.Exp, bias=mx[:, :1],
                                     accum_out=den[:, :1])
                rden = SCR.tile([128, 1], F32, tag="rden")
                nc.vector.reciprocal(rden[:], den[:])
                st = W6.tile([128, L], F32R, tag="S")
                nc.vector.tensor_scalar_mul(st[:], att[:], rden[:, :1])
                S.append(st)

            # ST[ti] = S^T
            ST = []
            for ti, (t0, t1) in enumerate(T_SL):
                t = W6.tile([128, L], F32R, tag="ST")
                for mi, (m0, m1) in enumerate(T_SL):
                    pt = PST.tile([128, 128], F32R, tag="psT")
                    nc.tensor.transpose(pt[:, :], S[mi][:, t0:t1], identr[:])
                    nc.vector.tensor_copy(t[:, m0:m1], pt[:, :])
                ST.append(t)

            # xpT[di] = sum_ti e_n[ti][:, d]^T @ ST[ti]
            xpT = []
            for di, (d0, d1) in enumerate(D_SL):
                dsz = d1 - d0
                ps = PS256.tile([128, L], F32, tag="ps256")
                for ti in range(2):
                    nc.tensor.matmul(ps[:dsz, :], lhsT=e_n[ti][:, d0:d1],
                                     rhs=ST[ti][:, :], start=(ti == 0), stop=(ti == 1))
                t = W8.tile([128, L], F32R, tag="xpT")
                nc.scalar.copy(t[:dsz, :], ps[:dsz, :])
                xpT.append(t)

            # pT = Wp^T [eT; xpT] + bp (linear)
            hT = eT + xpT
            pT = []
            for pi, (p0, p1) in enumerate(P_SL):
                psz = p1 - p0
                ps = PS256.tile([128, L], F32, tag="ps256")
                for k in range(6):
                    ksz = D_SL[k % 3][1] - D_SL[k % 3][0]
                    nc.tensor.matmul(ps[:psz, :], lhsT=wp_k[k][:ksz, p0:p1],
                                     rhs=hT[k][:ksz, :], start=(k == 0), stop=(k == 5))
                t = W8.tile([128, L], F32R, tag="pT")
                nc.scalar.activation(t[:psz, :], ps[:psz, :], AF.Identity,
                                     bias=bp_t[pi][:psz, :1])
                pT.append(t)

            # pRow[ti]: [128, 200] via transposes of pT
            pRow = []
            for ti, (t0, t1) in enumerate(T_SL):
                t = W8.tile([128, PROJ], F32R, tag="pRow")
                for pi, (p0, p1) in enumerate(P_SL):
                    psz = p1 - p0
                    pt = PST.tile([128, 128], F32R, tag="psT")
                    nc.tensor.transpose(pt[:, :psz], pT[pi][:psz, t0:t1],
                                        identr[:psz, :psz])
                    nc.vector.tensor_copy(t[:, p0:p1], pt[:, :psz])
                pRow.append(t)

            # aT = relu(Wa^T pT + ba)
            aT = []
            for ai, (a0, a1) in enumerate(P_SL):
                asz = a1 - a0
                ps = PS256.tile([128, L], F32, tag="ps256")
                for ki, (k0, k1) in enumerate(P_SL):
                    ksz = k1 - k0
                    nc.tensor.matmul(ps[:asz, :], lhsT=wa_k[ki][:ksz, a0:a1],
                                     rhs=pT[ki][:ksz, :], start=(ki == 0), stop=(ki == 1))
                t = W8.tile([128, L], F32R, tag="aT")
                nc.scalar.activation(t[:asz, :], ps[:asz, :], AF.Relu,
                                     bias=ba_t[ai][:asz, :1])
                aT.append(t)
            return pT, pRow, aT

        def rowsoftmax(src, dst_tag, pool):
            mx = SCR.tile([128, 1], F32, tag="mx")
            nc.vector.tensor_reduce(mx[:], src[:], axis=AX.X, op=ALU.max, negate=True)
            den = SCR.tile([128, 1], F32, tag="den")
            nc.scalar.activation(src[:], src[:], AF.Exp, bias=mx[:, :1],
                                 accum_out=den[:, :1])
            rden = SCR.tile([128, 1], F32, tag="rden")
            nc.vector.reciprocal(rden[:], den[:])
            t = pool.tile([128, L], F32R, tag=dst_tag)
            nc.vector.tensor_scalar_mul(t[:], src[:], rden[:, :1])
            return t

        for it in range(NIT):
            p1T, p1R, a1T = intra(1, it)
            p2T, p2R, a2T = intra(2, it)

            M1b = W4.tile([128, L], F32, tag="Mb")
            nc.vector.tensor_scalar(M1b[:], iotaB[:], sizebc[1][:, it:it + 1], None,
                                    op0=ALU.is_lt)
            M2b = W4.tile([128, L], F32, tag="Mb")
            nc.vector.tensor_scalar(M2b[:], iotaB[:], sizebc[2][:, it:it + 1], None,
                                    op0=ALU.is_lt)

            # sim = a1^T a2, masked
            sim = []
            for mi, (m0, m1) in enumerate(T_SL):
                ps = PS256.tile([128, L], F32, tag="ps256")
                for ai, (a0, a1) in enumerate(P_SL):
                    asz = a1 - a0
                    nc.tensor.matmul(ps[:, :], lhsT=a1T[ai][:asz, m0:m1],
                                     rhs=a2T[ai][:asz, :],
                                     start=(ai == 0), stop=(ai == 1))
                t = W4.tile([128, L], F32R, tag="sim")
                nc.vector.scalar_tensor_tensor(
                    t[:], ps[:], mcol[1][mi][:, it:it + 1], M2b[:],
                    op0=ALU.mult, op1=ALU.mult)
                sim.append(t)

            # simT
            simT = []
            for ti, (t0, t1) in enumerate(T_SL):
                t = W4.tile([128, L], F32, tag="simT")
                for mi, (m0, m1) in enumerate(T_SL):
                    pt = PST.tile([128, 128], F32R, tag="psT")
                    nc.tensor.transpose(pt[:, :], sim[mi][:, t0:t1], identr[:])
                    nc.vector.tensor_copy(t[:, m0:m1], pt[:, :])
                simT.append(t)

            Srow = [rowsoftmax(s, "S2", W6) for s in sim]
            NS = [rowsoftmax(s, "S2", W6) for s in simT]

            SrowT, NT = [], []
            for ti, (t0, t1) in enumerate(T_SL):
                a = W6.tile([128, L], F32R, tag="ST2")
                b = W6.tile([128, L], F32R, tag="ST2")
                for mi, (m0, m1) in enumerate(T_SL):
                    pt = PST.tile([128, 128], F32R, tag="psT")
                    nc.tensor.transpose(pt[:, :], Srow[mi][:, t0:t1], identr[:])
                    nc.vector.tensor_copy(a[:, m0:m1], pt[:, :])
                    pt2 = PST.tile([128, 128], F32R, tag="psT")
                    nc.tensor.transpose(pt2[:, :], NS[mi][:, t0:t1], identr[:])
                    nc.vector.tensor_copy(b[:, m0:m1], pt2[:, :])
                SrowT.append(a)
                NT.append(b)

            betaT, alphaT = [], []
            for pi, (p0, p1) in enumerate(P_SL):
                psz = p1 - p0
                ps = PS256.tile([128, L], F32, tag="ps256")
                for ki in range(2):
                    nc.tensor.matmul(ps[:psz, :], lhsT=p2R[ki][:, p0:p1],
                                     rhs=SrowT[ki][:, :], start=(ki == 0), stop=(ki == 1))
                t = W6.tile([128, L], F32R, tag="bT")
                nc.scalar.copy(t[:psz, :], ps[:psz, :])
                betaT.append(t)
                ps2 = PS256.tile([128, L], F32, tag="ps256")
                for ki in range(2):
                    nc.tensor.matmul(ps2[:psz, :], lhsT=p1R[ki][:, p0:p1],
                                     rhs=NT[ki][:, :], start=(ki == 0), stop=(ki == 1))
                t2 = W6.tile([128, L], F32R, tag="bT")
                nc.scalar.copy(t2[:psz, :], ps2[:psz, :])
                alphaT.append(t2)

            # compare + pool
            for s, pTt, oT, Mb in ((1, p1T, betaT, M1b), (2, p2T, alphaT, M2b)):
                kt = pTt + oT   # K-tiles sized 128,72,128,72
                for vi, (v0, v1) in enumerate(V_SL):
                    vsz = v1 - v0
                    ps = PS256.tile([128, L], F32, tag="ps256")
                    for k in range(4):
                        ksz = WC_K[k][1] - WC_K[k][0]
                        nc.tensor.matmul(ps[:vsz, :], lhsT=wc_k[k][:ksz, v0:v1],
                                         rhs=kt[k][:ksz, :],
                                         start=(k == 0), stop=(k == 3))
                    vt = W10.tile([128, L], F32, tag="vT")
                    nc.scalar.activation(vt[:vsz, :], ps[:vsz, :], AF.Relu,
                                         bias=bc_t[vi][:vsz, :1])
                    scr = SCR.tile([128, L], F32, tag="ttr")
                    nc.vector.scalar_tensor_tensor(
                        out=scr[:vsz, :], in0=vt[:vsz, :], scalar=1.0,
                        in1=Mb[:vsz, :], op0=ALU.mult, op1=ALU.mult,
                        accum_out=pooled[(s, vi)][:vsz, it:it + 1])

        # ---- aggregate all items: out = Wg^T pooled + bg
        pool_r = []
        for s in (1, 2):
            for vi, (v0, v1) in enumerate(V_SL):
                vsz = v1 - v0
                t = C.tile([128, NIT], F32R, tag=f"poolr{s}_{vi}")
                nc.vector.tensor_copy(t[:vsz, :], pooled[(s, vi)][:vsz, :])
                pool_r.append((t, vsz))
        psA = PSA.tile([CLS, NIT], F32, tag="agg")
        for k, (t, ksz) in enumerate(pool_r):
            nc.tensor.matmul(psA[:, :], lhsT=wg_k[k][:ksz, :], rhs=t[:ksz, :],
                             start=(k == 0), stop=(k == 7))
        out_sb = C.tile([CLS, NIT], F32, tag="outsb")
        nc.scalar.activation(out_sb[:], psA[:], AF.Identity, bias=bg_t[:CLS, :1])
        nc.sync.dma_start(out=out_d.ap(), in_=out_sb[:])


def _get_nc():
    global _CACHED_NC
    if _CACHED_NC is None:
        _CACHED_NC = _build_nc()
    return _CACHED_NC


def make_in_maps(inputs):
    x1 = np.asarray(inputs["x1"])
    x2 = np.asarray(inputs["x2"])
    f32 = lambda k: np.ascontiguousarray(np.asarray(inputs[k], dtype=np.float32))
    emb = f32("emb")
    col = lambda k: f32(k).reshape(-1, 1)
    ii, jj = np.meshgrid(np.arange(L), np.arange(L), indexing="ij")
    dmask = (np.abs(ii - jj) >= 10).astype(np.float32)
    bdist = np.full((128, 1), np.asarray(inputs["b_dist"], np.float32).reshape(-1)[0],
                    np.float32)

    shared = {
        "emb": emb,
        "wi": f32("Wi"), "bi": col("bi"),
        "wp": f32("Wp"), "bp": col("bp"),
        "wa": f32("Wa"), "ba": col("ba"),
        "wc": f32("Wc"), "bc": col("bc"),
        "wg": f32("Wg"), "bg": col("bg"),
        "bdist": bdist, "dmask": dmask,
    }
    in_maps = []
    for c in range(NCORES):
        sl = slice(c * NIT, (c + 1) * NIT)
        x1s = np.ascontiguousarray(x1[sl]).astype(np.int32)
        x2s = np.ascontiguousarray(x2[sl]).astype(np.int32)
        m = dict(shared)
        m["idx1"] = np.ascontiguousarray(x1s.reshape(-1).reshape(2 * NIT, 128).T)
        m["idx2"] = np.ascontiguousarray(x2s.reshape(-1).reshape(2 * NIT, 128).T)
        m["xi1"] = x1s
        m["xi2"] = x2s
        in_maps.append(m)
    return in_maps


def kernel(**inputs):
    nc = _get_nc()
    in_maps = make_in_maps(inputs)
    res = run_bass_kernel_spmd(nc, in_maps, core_ids=list(range(NCORES)))
    out = np.concatenate([r["out"].T for r in res.results], axis=0)
    return np.ascontiguousarray(out, dtype=np.float32)



# revision 3
# speedup vs baseline: 1.3561x; 1.3561x over previous
"""Trainium2 Bass kernel for nn_Decomposeable (decomposable attention model).

Strategy: data-parallel over batch B=128 across 8 NeuronCores (16 items/core,
processed as 8 pairs with free-dim-512 matmuls for all shared-weight FCs).
All matmul operands bf16 (fp32 PSUM accumulate). Softmax is transpose-free:
the intra attention matrix is symmetric, so its transposed softmax comes from
the same exp tiles scaled by a column-broadcast reciprocal denominator; for
the cross attention both sim and sim^T are computed by direct matmuls and
normalized the same way. Sequence masks fold into the exp scale (column) and
the attended operand (row mask), and pooling runs on the PE with the mask
column as lhsT. Scalar-engine functions are restricted to one activation
table set (exp/relu/identity/copy/square, plus grouped early sqrts).
"""
import sys
import numpy as np

for _p in ("/opt/trn_rl_repo",):
    if _p not in sys.path:
        sys.path.append(_p)

import ml_dtypes
import concourse.bass as bass
import concourse.bacc as bacc
import concourse.tile as tile
from concourse import mybir
from concourse.bass_utils import run_bass_kernel_spmd
from concourse.masks import make_identity

F32 = mybir.dt.float32
BF16 = mybir.dt.bfloat16
I32 = mybir.dt.int32
AF = mybir.ActivationFunctionType
ALU = mybir.AluOpType
AX = mybir.AxisListType
BF_NP = ml_dtypes.bfloat16

L, EMB, PROJ, ATT, CLS = 256, 300, 200, 200, 3
B, NCORES = 128, 8
NIT = B // NCORES            # items per core
NPAIR = NIT // 2
VOCAB = 50000

D_SL = [(0, 128), (128, 256), (256, 300)]          # EMB k-tiles
A_SL = [(0, 128), (128, 200)]                      # ATT/PROJ tiles
WC_K = [(0, 128), (128, 200), (200, 328), (328, 400)]
V_CH = [(0, 128), (128, 256), (256, 384), (384, 400)]  # P transpose chunks

_CACHED_NC = None


def _build_nc():
    nc = bacc.Bacc("TRN2", target_bir_lowering=False, debug=False)

    dram = {}
    def din(name, shape, dt):
        dram[name] = nc.dram_tensor(name, shape, dt, kind="ExternalInput")
        return dram[name]

    din("idx1", [128, 2 * NIT], I32)
    din("idx2", [128, 2 * NIT], I32)
    din("xi1", [NIT, L], I32)
    din("xi2", [NIT, L], I32)
    din("emb", [VOCAB, EMB], BF16)
    din("wi", [EMB, ATT], BF16)
    din("wp", [2 * EMB, PROJ], BF16)
    din("wa", [PROJ, ATT], BF16)
    din("wc", [2 * PROJ, 2 * PROJ], BF16)
    din("wg", [4 * PROJ, CLS], BF16)
    din("bi", [ATT, 1], F32)
    din("bp", [PROJ, 1], F32)
    din("ba_row", [1, ATT], BF16)
    din("bc_row", [1, 2 * PROJ], BF16)
    din("bg_row", [1, CLS], BF16)
    din("dmask", [L, L], BF16)
    din("bdist", [128, 1], F32)
    out_d = nc.dram_tensor("out", [CLS, NIT], F32, kind="ExternalOutput")

    with tile.TileContext(nc) as tc:
        _emit(nc, tc, dram, out_d)
    nc.compile()
    return nc


def _emit(nc, tc, dram, out_d):
    from contextlib import ExitStack
    ctx = ExitStack()
    with ctx:
        C = ctx.enter_context(tc.tile_pool(name="consts", bufs=1))
        PS = ctx.enter_context(tc.tile_pool(name="ps", bufs=7, space="PSUM"))
        W = ctx.enter_context(tc.tile_pool(name="work", bufs=2))

        def ps_tile(shape, dt=F32):
            return PS.tile(shape, dt, tag="ps", name="ps")

        # ---------------- constants ----------------
        ident_f = C.tile([128, 128], F32)
        make_identity(nc, ident_f[:])
        ident = C.tile([128, 128], BF16)
        nc.vector.tensor_copy(ident[:], ident_f[:])
        ones_bf = C.tile([1, 512], BF16)
        nc.vector.memset(ones_bf[:], 1.0)
        iota_i = C.tile([NIT, L], I32)
        nc.gpsimd.iota(iota_i[:], pattern=[[1, L]], base=0, channel_multiplier=0)
        iota16 = C.tile([NIT, L], F32)
        nc.vector.tensor_copy(iota16[:], iota_i[:])

        def load(name, r0, r1, dt=BF16):
            src = dram[name].ap()
            w = src.shape[1]
            t = C.tile([128, w], dt, tag=f"{name}_{r0}", name=f"{name}_{r0}")
            nc.sync.dma_start(out=t[:r1 - r0, :], in_=src[r0:r1, :])
            return t

        wi_k = [load("wi", d0, d1) for (d0, d1) in D_SL]
        wp_k = [load("wp", d0, d1) for (d0, d1) in D_SL] + \
               [load("wp", 300 + d0, 300 + d1) for (d0, d1) in D_SL]
        wa_k = [load("wa", a0, a1) for (a0, a1) in A_SL]
        wc_k = [load("wc", k0, k1) for (k0, k1) in WC_K]
        wg_k = [load("wg", s * 400 + v0, s * 400 + v1)
                for s in range(2) for (v0, v1) in V_CH]
        bi_sl = [load("bi", a0, a1, F32) for (a0, a1) in A_SL]
        bp_sl = [load("bp", p0, p1, F32) for (p0, p1) in A_SL]
        ba_row = load("ba_row", 0, 1)
        bc_row = load("bc_row", 0, 1)
        bg_row = load("bg_row", 0, 1)
        dmask_sb = [load("dmask", t0, t1) for (t0, t1) in [(0, 128), (128, 256)]]
        bdist = load("bdist", 0, 128, F32)

        idx_sb = {}
        for s, name in ((0, "idx1"), (1, "idx2")):
            t = C.tile([128, 2 * NIT], I32, tag=name, name=name)
            nc.sync.dma_start(out=t[:], in_=dram[name].ap())
            idx_sb[s] = t

        # bias2d pair tiles: [x-blk, 512] f32 = bdist * (dist>=10), duplicated
        bias2d = []
        for xb in range(2):
            b2 = C.tile([128, 512], F32, tag=f"bias2d_{xb}", name=f"bias2d_{xb}")
            for h in range(2):
                nc.vector.tensor_scalar_mul(
                    b2[:, h * 256:(h + 1) * 256], dmask_sb[xb][:], bdist[:, :1])
            bias2d.append(b2)

        # ---------------- masks ----------------
        m_bf = {}
        mcol_f = {}
        mcol_b = {}
        for s, name in ((0, "xi1"), (1, "xi2")):
            xi = C.tile([NIT, L], I32, tag=name, name=name)
            nc.sync.dma_start(out=xi[:], in_=dram[name].ap())
            xf = W.tile([NIT, L], F32, tag="xf", name="xf")
            nc.vector.tensor_copy(xf[:], xi[:])
            nz = W.tile([NIT, L], F32, tag="nz", name="nz")
            nc.vector.tensor_scalar(nz[:], xf[:], 0.0, None, op0=ALU.not_equal)
            sizes = C.tile([NIT, 1], F32, tag=f"sizes{s}", name=f"sizes{s}")
            nc.vector.tensor_reduce(sizes[:], nz[:], axis=AX.X, op=ALU.add)
            mb = C.tile([NIT, L], BF16, tag=f"mbf{s}", name=f"mbf{s}")
            nc.vector.tensor_scalar(mb[:], iota16[:], sizes[:, :1], None,
                                    op0=ALU.is_lt)
            m_bf[s] = mb
            cf, cb = [], []
            for ti in range(2):
                pt = ps_tile([128, NIT], BF16)
                nc.tensor.transpose(pt[:, :NIT], mb[:NIT, ti * 128:(ti + 1) * 128],
                                    ident[:NIT, :NIT])
                f = C.tile([128, NIT], F32, tag=f"mcf{s}_{ti}", name=f"mcf{s}_{ti}")
                nc.vector.tensor_copy(f[:], pt[:, :NIT])
                bt = C.tile([128, NIT], BF16, tag=f"mcb{s}_{ti}", name=f"mcb{s}_{ti}")
                nc.scalar.copy(bt[:], pt[:, :NIT])
                cf.append(f)
                cb.append(bt)
            mcol_f[s] = cf
            mcol_b[s] = cb

        # maskrow pair tiles [128, 512] bf16 per (side, pair)
        maskrow = {}
        for s in range(2):
            for p in range(NPAIR):
                rps = ps_tile([1, 512], BF16)
                for h in range(2):
                    it = 2 * p + h
                    for ti in range(2):
                        nc.tensor.transpose(
                            rps[:1, h * 256 + ti * 128: h * 256 + (ti + 1) * 128],
                            mcol_b[s][ti][:, it:it + 1], ident[:])
                mrow = W.tile([1, 512], BF16, tag="mrow", name="mrow", bufs=4)
                nc.scalar.copy(mrow[:], rps[:1, :])
                bps = ps_tile([128, 512])
                nc.tensor.matmul(bps[:, :], lhsT=ones_bf[:1, :128],
                                 rhs=mrow[:1, :], start=True, stop=True)
                mr = C.tile([128, 512], BF16, tag=f"mrow{s}_{p}", name=f"mrow{s}_{p}")
                nc.vector.tensor_copy(mr[:], bps[:, :])
                maskrow[(s, p)] = mr

        # ---------------- embedding gathers + norms ----------------
        # e tiles [128, 600] bf16 per (side, block, pair): [item A | item B]
        e_n = {}
        for p in range(NPAIR):
            for s in range(2):
                for ti in range(2):
                    t = C.tile([128, 600], BF16, tag=f"e_{s}_{ti}_{p}",
                               name=f"e_{s}_{ti}_{p}")
                    e_n[(s, ti, p)] = t
                    for h in range(2):
                        g = 2 * (2 * p + h) + ti
                        nc.gpsimd.indirect_dma_start(
                            out=t[:, h * 300:(h + 1) * 300], out_offset=None,
                            in_=dram["emb"].ap(),
                            in_offset=bass.IndirectOffsetOnAxis(
                                ap=idx_sb[s][:, g:g + 1], axis=0))

        invsq = C.tile([128, 8 * NPAIR], F32)
        nrm = C.tile([128, 8 * NPAIR], F32)
        inv = C.tile([128, 8 * NPAIR], F32)

        def gcol(p, s, ti, h):
            return 8 * p + 4 * s + 2 * ti + h

        def emit_sq(p):
            for s in range(2):
                for ti in range(2):
                    for h in range(2):
                        scr = W.tile([128, 300], BF16, tag="sqscr", name="sqscr",
                                     bufs=3)
                        nc.scalar.activation(
                            scr[:], e_n[(s, ti, p)][:, h * 300:(h + 1) * 300],
                            AF.Square, accum_out=invsq[:, gcol(p, s, ti, h):
                                                       gcol(p, s, ti, h) + 1])

        def emit_norm_batch(c0, c1, pairs):
            nc.scalar.activation(nrm[:, c0:c1], invsq[:, c0:c1], AF.Sqrt)
            nc.vector.reciprocal(inv[:, c0:c1], nrm[:, c0:c1])
            for p in pairs:
                for s in range(2):
                    for ti in range(2):
                        for h in range(2):
                            g = gcol(p, s, ti, h)
                            t = e_n[(s, ti, p)]
                            nc.vector.tensor_scalar_mul(
                                t[:, h * 300:(h + 1) * 300],
                                t[:, h * 300:(h + 1) * 300], inv[:, g:g + 1])

        emit_sq(0)
        emit_norm_batch(0, 8, [0])
        emit_sq(1)
        emit_norm_batch(8, 16, [1])
        for p in (2, 3):
            emit_sq(p)
        emit_norm_batch(16, 32, [2, 3])
        for p in (4, 5, 6, 7):
            emit_sq(p)
        emit_norm_batch(32, 64, [4, 5, 6, 7])

        # pooled destination rows (filled by sbuf-to-sbuf DMA from row tiles)
        P_f = [C.tile([NIT, 400], F32, tag=f"P{s}", name=f"P{s}") for s in range(2)]

        # ---------------- per-pair main pipeline ----------------
        def wtile(tag, shape=(128, 512), dt=BF16, bufs=2):
            return W.tile(list(shape), dt, tag=tag, name=tag, bufs=bufs)

        def rden_broadcast(denst, prefix):
            """denst [128,4] f32 cols (h,blk) -> psum [128,512] f32 of
            reciprocal denominators broadcast down partitions."""
            rden = wtile(f"{prefix}_rd", (128, 4), F32, bufs=4)
            nc.vector.reciprocal(rden[:], denst[:])
            rdbf = wtile(f"{prefix}_rdb", (128, 4), BF16, bufs=4)
            nc.vector.tensor_copy(rdbf[:], rden[:])
            rowps = ps_tile([1, 512], BF16)
            for h in range(2):
                for blk in range(2):
                    c = 2 * h + blk
                    nc.tensor.transpose(
                        rowps[:1, h * 256 + blk * 128: h * 256 + (blk + 1) * 128],
                        rdbf[:, c:c + 1], ident[:])
            rrow = wtile(f"{prefix}_rr", (1, 512), BF16, bufs=4)
            nc.scalar.copy(rrow[:], rowps[:1, :])
            bps = ps_tile([128, 512])
            nc.tensor.matmul(bps[:, :], lhsT=ones_bf[:1, :128], rhs=rrow[:1, :],
                             start=True, stop=True)
            return bps

        def intra(s, p):
            """Returns (eT, pT, pRow, aT) tile lists for side s of pair p."""
            # eT[di]: [dsz, 512] via transposes of e_n
            eT = []
            for di, (d0, d1) in enumerate(D_SL):
                dsz = d1 - d0
                tps = ps_tile([128, 512], BF16)
                for h in range(2):
                    for ti in range(2):
                        nc.tensor.transpose(
                            tps[:dsz, h * 256 + ti * 128: h * 256 + (ti + 1) * 128],
                            e_n[(s, ti, p)][:, h * 300 + d0: h * 300 + d1],
                            ident[:])
                t = wtile(f"eT{di}")
                nc.vector.tensor_copy(t[:dsz, :], tps[:dsz, :])
                eT.append(t)

            # fT = relu(Wi^T eT + bi)
            fT = []
            for ai, (a0, a1) in enumerate(A_SL):
                asz = a1 - a0
                ps = ps_tile([128, 512])
                for k in range(3):
                    ksz = D_SL[k][1] - D_SL[k][0]
                    nc.tensor.matmul(ps[:asz, :], lhsT=wi_k[k][:ksz, a0:a1],
                                     rhs=eT[k][:ksz, :], start=(k == 0),
                                     stop=(k == 2))
                t = wtile(f"fT{ai}")
                nc.scalar.activation(t[:asz, :], ps[:asz, :], AF.Relu,
                                     bias=bi_sl[ai][:asz, :1])
                fT.append(t)

            # att tiles + bias2d, exp with accumulated denominators
            denst = wtile("iden", (128, 4), F32, bufs=4)
            E = []
            for xb in range(2):
                ps = ps_tile([128, 512])
                for h in range(2):
                    for ai, (a0, a1) in enumerate(A_SL):
                        asz = a1 - a0
                        nc.tensor.matmul(
                            ps[:, h * 256:(h + 1) * 256],
                            lhsT=fT[ai][:asz, h * 256 + xb * 128:
                                        h * 256 + (xb + 1) * 128],
                            rhs=fT[ai][:asz, h * 256:(h + 1) * 256],
                            start=(ai == 0), stop=(ai == 1))
                att_s = wtile(f"atts{xb}", (128, 512), F32)
                nc.vector.tensor_add(att_s[:], ps[:], bias2d[xb][:])
                et = wtile(f"E{xb}")
                for h in range(2):
                    nc.scalar.activation(
                        et[:, h * 256:(h + 1) * 256],
                        att_s[:, h * 256:(h + 1) * 256], AF.Exp,
                        accum_out=denst[:, 2 * h + xb: 2 * h + xb + 1])
                E.append(et)

            bps = rden_broadcast(denst, "i")
            ST = []
            for blk in range(2):
                t = wtile(f"ST{blk}")
                nc.vector.tensor_mul(t[:], E[blk][:], bps[:, :])
                ST.append(t)

            # xpT[di] [dsz, 512]
            xpT = []
            for di, (d0, d1) in enumerate(D_SL):
                dsz = d1 - d0
                ps = ps_tile([128, 512])
                for h in range(2):
                    for ti in range(2):
                        nc.tensor.matmul(
                            ps[:dsz, h * 256:(h + 1) * 256],
                            lhsT=e_n[(s, ti, p)][:, h * 300 + d0: h * 300 + d1],
                            rhs=ST[ti][:, h * 256:(h + 1) * 256],
                            start=(ti == 0), stop=(ti == 1))
                t = wtile(f"xp{di}")
                nc.scalar.copy(t[:dsz, :], ps[:dsz, :])
                xpT.append(t)

            # pT = Wp^T [eT; xpT] + bp
            hT = eT + xpT
            pT = []
            for pi, (p0, p1) in enumerate(A_SL):
                psz = p1 - p0
                ps = ps_tile([128, 512])
                for k in range(6):
                    ksz = D_SL[k % 3][1] - D_SL[k % 3][0]
                    nc.tensor.matmul(ps[:psz, :], lhsT=wp_k[k][:ksz, p0:p1],
                                     rhs=hT[k][:ksz, :], start=(k == 0),
                                     stop=(k == 5))
                t = wtile(f"pT{pi}")
                nc.scalar.activation(t[:psz, :], ps[:psz, :], AF.Identity,
                                     bias=bp_sl[pi][:psz, :1])
                pT.append(t)

            # pRow[ti] [128, 400] = [item A p-dims | item B p-dims]
            pRow = []
            for ti in range(2):
                tps = ps_tile([128, 400], BF16)
                for h in range(2):
                    for pi, (p0, p1) in enumerate(A_SL):
                        psz = p1 - p0
                        nc.tensor.transpose(
                            tps[:, h * 200 + p0: h * 200 + p1],
                            pT[pi][:psz, h * 256 + ti * 128:
                                   h * 256 + (ti + 1) * 128],
                            ident[:psz, :psz])
                t = wtile(f"pR{ti}", (128, 400))
                nc.scalar.copy(t[:], tps[:, :])
                pRow.append(t)

            # aT = relu(Wa^T pT + ba) * maskrow  (row-masked)
            aT = []
            for ai, (a0, a1) in enumerate(A_SL):
                asz = a1 - a0
                ps = ps_tile([128, 512])
                for ki, (k0, k1) in enumerate(A_SL):
                    ksz = k1 - k0
                    nc.tensor.matmul(ps[:asz, :], lhsT=wa_k[ki][:ksz, a0:a1],
                                     rhs=pT[ki][:ksz, :], start=(ki == 0),
                                     stop=False)
                nc.tensor.matmul(ps[:asz, :], lhsT=ba_row[:1, a0:a1],
                                 rhs=ones_bf[:1, :], start=False, stop=True)
                t = wtile(f"aT{ai}")
                nc.vector.scalar_tensor_tensor(
                    t[:asz, :], ps[:asz, :], 0.0, maskrow[(s, p)][:asz, :],
                    op0=ALU.max, op1=ALU.mult)
                aT.append(t)
            return eT, pT, pRow, aT

        for p in range(NPAIR):
            eT1, p1T, p1R, a1T = intra(0, p)
            eT2, p2T, p2R, a2T = intra(1, p)

            # sim and simT via direct matmuls; exp with column mask as scale
            den2 = wtile("den2", (128, 4), F32, bufs=4)
            den1 = wtile("den1", (128, 4), F32, bufs=4)
            E2, E1 = [], []
            for xb in range(2):
                ps = ps_tile([128, 512])
                for h in range(2):
                    for ai, (a0, a1) in enumerate(A_SL):
                        asz = a1 - a0
                        nc.tensor.matmul(
                            ps[:, h * 256:(h + 1) * 256],
                            lhsT=a1T[ai][:asz, h * 256 + xb * 128:
                                         h * 256 + (xb + 1) * 128],
                            rhs=a2T[ai][:asz, h * 256:(h + 1) * 256],
                            start=(ai == 0), stop=(ai == 1))
                et = wtile(f"E2_{xb}")
                for h in range(2):
                    it = 2 * p + h
                    nc.scalar.activation(
                        et[:, h * 256:(h + 1) * 256],
                        ps[:, h * 256:(h + 1) * 256], AF.Exp,
                        scale=mcol_f[0][xb][:, it:it + 1],
                        accum_out=den2[:, 2 * h + xb: 2 * h + xb + 1])
                E2.append(et)
            for yb in range(2):
                ps = ps_tile([128, 512])
                for h in range(2):
                    for ai, (a0, a1) in enumerate(A_SL):
                        asz = a1 - a0
                        nc.tensor.matmul(
                            ps[:, h * 256:(h + 1) * 256],
                            lhsT=a2T[ai][:asz, h * 256 + yb * 128:
                                         h * 256 + (yb + 1) * 128],
                            rhs=a1T[ai][:asz, h * 256:(h + 1) * 256],
                            start=(ai == 0), stop=(ai == 1))
                et = wtile(f"E1_{yb}")
                for h in range(2):
                    it = 2 * p + h
                    nc.scalar.activation(
                        et[:, h * 256:(h + 1) * 256],
                        ps[:, h * 256:(h + 1) * 256], AF.Exp,
                        scale=mcol_f[1][yb][:, it:it + 1],
                        accum_out=den1[:, 2 * h + yb: 2 * h + yb + 1])
                E1.append(et)

            b1 = rden_broadcast(den1, "x1")
            S1 = []
            for xb in range(2):
                t = wtile(f"S1_{xb}")
                nc.vector.tensor_mul(t[:], E2[xb][:], b1[:, :])
                S1.append(t)
            b2 = rden_broadcast(den2, "x2")
            S2T = []
            for yb in range(2):
                t = wtile(f"S2T_{yb}")
                nc.vector.tensor_mul(t[:], E1[yb][:], b2[:, :])
                S2T.append(t)

            # betaT = p2R^T-contract S2T ; alphaT = p1R-contract S1
            betaT, alphaT = [], []
            for pi, (p0, p1) in enumerate(A_SL):
                psz = p1 - p0
                ps = ps_tile([128, 512])
                for h in range(2):
                    for ti in range(2):
                        nc.tensor.matmul(
                            ps[:psz, h * 256:(h + 1) * 256],
                            lhsT=p2R[ti][:, h * 200 + p0: h * 200 + p1],
                            rhs=S2T[ti][:, h * 256:(h + 1) * 256],
                            start=(ti == 0), stop=(ti == 1))
                t = wtile(f"bT{pi}")
                nc.scalar.copy(t[:psz, :], ps[:psz, :])
                betaT.append(t)
            for pi, (p0, p1) in enumerate(A_SL):
                psz = p1 - p0
                ps = ps_tile([128, 512])
                for h in range(2):
                    for xb in range(2):
                        nc.tensor.matmul(
                            ps[:psz, h * 256:(h + 1) * 256],
                            lhsT=p1R[xb][:, h * 200 + p0: h * 200 + p1],
                            rhs=S1[xb][:, h * 256:(h + 1) * 256],
                            start=(xb == 0), stop=(xb == 1))
                t = wtile(f"alT{pi}")
                nc.scalar.copy(t[:psz, :], ps[:psz, :])
                alphaT.append(t)

            # compare (row layout) + PE pooling
            for s, pTt, oT in ((0, p1T, betaT), (1, p2T, alphaT)):
                kt = pTt + oT
                for h in range(2):
                    it = 2 * p + h
                    vrs = []
                    for ti in range(2):
                        cps = ps_tile([128, 400])
                        for k in range(4):
                            ksz = WC_K[k][1] - WC_K[k][0]
                            nc.tensor.matmul(
                                cps[:, :],
                                lhsT=kt[k][:ksz, h * 256 + ti * 128:
                                           h * 256 + (ti + 1) * 128],
                                rhs=wc_k[k][:ksz, :400],
                                start=(k == 0), stop=False)
                        nc.tensor.matmul(cps[:, :], lhsT=ones_bf[:1, :128],
                                         rhs=bc_row[:1, :400],
                                         start=False, stop=True)
                        vr = wtile("vr", (128, 400), BF16, bufs=4)
                        nc.vector.tensor_scalar(vr[:], cps[:, :], 0.0, None,
                                                op0=ALU.max)
                        vrs.append(vr)
                    pps = ps_tile([1, 400])
                    for ti in range(2):
                        nc.tensor.matmul(pps[:1, :],
                                         lhsT=mcol_b[s][ti][:, it:it + 1],
                                         rhs=vrs[ti][:, :],
                                         start=(ti == 0), stop=(ti == 1))
                    prow = wtile("prow", (1, 400), F32, bufs=4)
                    if s == 0:
                        nc.scalar.copy(prow[:], pps[:1, :])
                    else:
                        nc.vector.tensor_copy(prow[:], pps[:1, :])
                    nc.sync.dma_start(out=P_f[s][it:it + 1, :], in_=prow[:1, :])

        # ---------------- aggregate ----------------
        PT_sb = []
        for s in range(2):
            pb = C.tile([NIT, 400], BF16, tag=f"Pb{s}", name=f"Pb{s}")
            nc.vector.tensor_copy(pb[:], P_f[s][:])
            for c, (c0, c1) in enumerate(V_CH):
                csz = c1 - c0
                tps = ps_tile([128, NIT], BF16)
                nc.tensor.transpose(tps[:csz, :NIT], pb[:NIT, c0:c1],
                                    ident[:NIT, :NIT])
                t = C.tile([128, NIT], BF16, tag=f"PT{s}_{c}", name=f"PT{s}_{c}")
                nc.scalar.copy(t[:csz, :], tps[:csz, :])
                PT_sb.append(t)
        aps = ps_tile([CLS, NIT])
        for k in range(8):
            ksz = V_CH[k % 4][1] - V_CH[k % 4][0]
            nc.tensor.matmul(aps[:, :], lhsT=wg_k[k][:ksz, :CLS],
                             rhs=PT_sb[k][:ksz, :], start=(k == 0), stop=False)
        nc.tensor.matmul(aps[:, :], lhsT=bg_row[:1, :CLS],
                         rhs=ones_bf[:1, :NIT], start=False, stop=True)
        out_sb = C.tile([CLS, NIT], F32)
        nc.scalar.copy(out_sb[:], aps[:, :])
        nc.sync.dma_start(out=out_d.ap(), in_=out_sb[:])


def _get_nc():
    global _CACHED_NC
    if _CACHED_NC is None:
        _CACHED_NC = _build_nc()
    return _CACHED_NC


def make_in_maps(inputs):
    x1 = np.asarray(inputs["x1"])
    x2 = np.asarray(inputs["x2"])
    f32 = lambda k: np.ascontiguousarray(np.asarray(inputs[k], dtype=np.float32))
    bf = lambda a: np.ascontiguousarray(np.asarray(a, dtype=np.float32)).astype(BF_NP)
    ii, jj = np.meshgrid(np.arange(L), np.arange(L), indexing="ij")
    dmask = (np.abs(ii - jj) >= 10).astype(np.float32)
    bdist = np.full((128, 1), np.asarray(inputs["b_dist"], np.float32).reshape(-1)[0],
                    np.float32)

    shared = {
        "emb": bf(inputs["emb"]),
        "wi": bf(inputs["Wi"]), "wp": bf(inputs["Wp"]), "wa": bf(inputs["Wa"]),
        "wc": bf(inputs["Wc"]), "wg": bf(inputs["Wg"]),
        "bi": f32("bi").reshape(-1, 1), "bp": f32("bp").reshape(-1, 1),
        "ba_row": bf(np.asarray(inputs["ba"]).reshape(1, -1)),
        "bc_row": bf(np.asarray(inputs["bc"]).reshape(1, -1)),
        "bg_row": bf(np.asarray(inputs["bg"]).reshape(1, -1)),
        "dmask": dmask.astype(BF_NP), "bdist": bdist,
    }
    in_maps = []
    for c in range(NCORES):
        sl = slice(c * NIT, (c + 1) * NIT)
        x1s = np.ascontiguousarray(x1[sl]).astype(np.int32)
        x2s = np.ascontiguousarray(x2[sl]).astype(np.int32)
        m = dict(shared)
        m["idx1"] = np.ascontiguousarray(x1s.reshape(-1).reshape(2 * NIT, 128).T)
        m["idx2"] = np.ascontiguousarray(x2s.reshape(-1).reshape(2 * NIT, 128).T)
        m["xi1"] = x1s
        m["xi2"] = x2s
        in_maps.append(m)
    return in_maps


def kernel(**inputs):
    nc = _get_nc()
    in_maps = make_in_maps(inputs)
    res = run_bass_kernel_spmd(nc, in_maps, core_ids=list(range(NCORES)))
    out = np.concatenate([r["out"].T for r in res.results], axis=0)
    return np.ascontiguousarray(out, dtype=np.float32)


# revision 11
# speedup vs baseline: 1.6016x; 1.1810x over previous
"""Trainium2 Bass kernel for nn_Decomposeable (decomposable attention model).

Strategy: data-parallel over batch B=128 across 8 NeuronCores (16 items/core,
processed as 8 pairs with free-dim-512 matmuls for all shared-weight FCs).
All matmul operands bf16 (fp32 PSUM accumulate). Softmax is transpose-free:
the intra attention matrix is symmetric and the cross attention is computed
in both orientations by direct matmuls; attention-weight matmuls consume the
raw exp tiles and the reciprocal softmax denominators are applied at PSUM
drain time via a ones-outer-product broadcast. Sequence masks fold into the
exp scale column and the attended operand rows; pooling runs on the PE with
the mask column as lhsT. Per-pair work is emitted as a 6-stage software
pipeline so every cross-engine latency is covered by other pairs' matmuls.
"""
import sys
import numpy as np

for _p in ("/opt/trn_rl_repo",):
    if _p not in sys.path:
        sys.path.append(_p)

import ml_dtypes
import concourse.bass as bass
import concourse.bacc as bacc
import concourse.tile as tile
from concourse import mybir
from concourse.bass_utils import run_bass_kernel_spmd
from concourse.masks import make_identity

F32 = mybir.dt.float32
BF16 = mybir.dt.bfloat16
I32 = mybir.dt.int32
AF = mybir.ActivationFunctionType
ALU = mybir.AluOpType
AX = mybir.AxisListType
BF_NP = ml_dtypes.bfloat16

L, EMB, PROJ, ATT, CLS = 256, 300, 200, 200, 3
B, NCORES = 128, 8
NIT = B // NCORES            # items per core
NPAIR = NIT // 2
VOCAB = 50000

D_SL = [(0, 128), (128, 256), (256, 300)]          # EMB k-tiles
A_SL = [(0, 128), (128, 200)]                      # ATT/PROJ tiles
WC_K = [(0, 128), (128, 200), (200, 328), (328, 400)]
V_CH = [(0, 128), (128, 256), (256, 384), (384, 400)]  # P transpose chunks

_CACHED_NC = None


def _build_nc():
    nc = bacc.Bacc("TRN2", target_bir_lowering=False, debug=False)

    dram = {}
    def din(name, shape, dt):
        dram[name] = nc.dram_tensor(name, shape, dt, kind="ExternalInput")
        return dram[name]

    din("idx1", [128, 2 * NIT], I32)
    din("idx2", [128, 2 * NIT], I32)
    din("xi1", [NIT, L], I32)
    din("xi2", [NIT, L], I32)
    din("emb", [VOCAB, EMB], BF16)
    din("wi", [EMB, ATT], BF16)
    din("wp", [2 * EMB, PROJ], BF16)
    din("wa", [PROJ, ATT], BF16)
    din("wc", [2 * PROJ, 2 * PROJ], BF16)
    din("wg", [4 * PROJ, CLS], BF16)
    din("bi", [ATT, 1], F32)
    din("bp", [PROJ, 1], F32)
    din("ba_row", [1, ATT], BF16)
    din("bc_row", [1, 2 * PROJ], BF16)
    din("bg_row", [1, CLS], BF16)
    din("dmask", [L, L], BF16)
    din("bdist", [128, 1], F32)
    out_d = nc.dram_tensor("out", [CLS, NIT], F32, kind="ExternalOutput")

    with tile.TileContext(nc) as tc:
        _emit(nc, tc, dram, out_d)
    nc.compile()
    return nc


def _emit(nc, tc, dram, out_d):
    from contextlib import ExitStack
    ctx = ExitStack()
    with ctx:
        C = ctx.enter_context(tc.tile_pool(name="consts", bufs=1))
        PS = ctx.enter_context(tc.tile_pool(name="ps", bufs=7, space="PSUM"))
        W = ctx.enter_context(tc.tile_pool(name="work", bufs=3))

        def ps_tile(shape, dt=F32):
            return PS.tile(shape, dt, tag="ps", name="ps")

        def wtile(tag, shape=(128, 512), dt=BF16, bufs=3):
            return W.tile(list(shape), dt, tag=tag, name=tag, bufs=bufs)

        # ---------------- small input DMAs first ----------------
        idx_sb = {}
        for s, name in ((0, "idx1"), (1, "idx2")):
            t = C.tile([128, 2 * NIT], I32, tag=name, name=name)
            nc.sync.dma_start(out=t[:], in_=dram[name].ap())
            idx_sb[s] = t
        xi_sb = {}
        for s, name in ((0, "xi1"), (1, "xi2")):
            t = C.tile([NIT, L], I32, tag=name, name=name)
            nc.sync.dma_start(out=t[:], in_=dram[name].ap())
            xi_sb[s] = t

        # ---------------- constants ----------------
        ident_f = C.tile([128, 128], F32)
        make_identity(nc, ident_f[:])
        ident = C.tile([128, 128], BF16)
        nc.vector.tensor_copy(ident[:], ident_f[:])
        ones_bf = C.tile([1, 512], BF16)
        nc.vector.memset(ones_bf[:], 1.0)
        iota_i = C.tile([NIT, L], I32)
        nc.gpsimd.iota(iota_i[:], pattern=[[1, L]], base=0, channel_multiplier=0)
        iota16 = C.tile([NIT, L], F32)
        nc.vector.tensor_copy(iota16[:], iota_i[:])

        # ---------------- embedding gathers + squares (gpsimd) ----------------
        e_n = {}
        invsq = C.tile([128, 8 * NPAIR], F32)
        inv = C.tile([128, 8 * NPAIR], F32)

        def gcol(p, s, ti, h):
            return 8 * p + 4 * s + 2 * ti + h

        for p in range(NPAIR):
            for s in range(2):
                for ti in range(2):
                    t = C.tile([128, 600], BF16, tag=f"e_{s}_{ti}_{p}",
                               name=f"e_{s}_{ti}_{p}")
                    e_n[(s, ti, p)] = t
                    for h in range(2):
                        g = 2 * (2 * p + h) + ti
                        nc.gpsimd.indirect_dma_start(
                            out=t[:, h * 300:(h + 1) * 300], out_offset=None,
                            in_=dram["emb"].ap(),
                            in_offset=bass.IndirectOffsetOnAxis(
                                ap=idx_sb[s][:, g:g + 1], axis=0))

        # ---------------- masks ----------------
        m_bf = {}
        mcol_f = {}
        mcol_b = {}
        for s in range(2):
            xf = W.tile([NIT, L], F32, tag="xf", name="xf", bufs=1)
            nc.vector.tensor_copy(xf[:], xi_sb[s][:])
            nz = W.tile([NIT, L], F32, tag="nz", name="nz", bufs=1)
            nc.vector.tensor_scalar(nz[:], xf[:], 0.0, None, op0=ALU.not_equal)
            sizes = C.tile([NIT, 1], F32, tag=f"sizes{s}", name=f"sizes{s}")
            nc.vector.tensor_reduce(sizes[:], nz[:], axis=AX.X, op=ALU.add)
            mb = C.tile([NIT, L], BF16, tag=f"mbf{s}", name=f"mbf{s}")
            nc.vector.tensor_scalar(mb[:], iota16[:], sizes[:, :1], None,
                                    op0=ALU.is_lt)
            m_bf[s] = mb
            cf, cb = [], []
            for ti in range(2):
                pt = ps_tile([128, NIT], BF16)
                nc.tensor.transpose(pt[:, :NIT], mb[:NIT, ti * 128:(ti + 1) * 128],
                                    ident[:NIT, :NIT])
                f = C.tile([128, NIT], F32, tag=f"mcf{s}_{ti}", name=f"mcf{s}_{ti}")
                nc.vector.tensor_copy(f[:], pt[:, :NIT])
                bt = C.tile([128, NIT], BF16, tag=f"mcb{s}_{ti}", name=f"mcb{s}_{ti}")
                nc.scalar.copy(bt[:], pt[:, :NIT])
                cf.append(f)
                cb.append(bt)
            mcol_f[s] = cf
            mcol_b[s] = cb

        # ---------------- weights ----------------
        def load(name, r0, r1, dt=BF16):
            src = dram[name].ap()
            w = src.shape[1]
            t = C.tile([128, w], dt, tag=f"{name}_{r0}", name=f"{name}_{r0}")
            nc.sync.dma_start(out=t[:r1 - r0, :], in_=src[r0:r1, :])
            return t

        wi_k = [load("wi", d0, d1) for (d0, d1) in D_SL]
        wp_k = [load("wp", d0, d1) for (d0, d1) in D_SL] + \
               [load("wp", 300 + d0, 300 + d1) for (d0, d1) in D_SL]
        wa_k = [load("wa", a0, a1) for (a0, a1) in A_SL]
        wc_k = [load("wc", k0, k1) for (k0, k1) in WC_K]
        wg_k = [load("wg", s * 400 + v0, s * 400 + v1)
                for s in range(2) for (v0, v1) in V_CH]
        bi_sl = [load("bi", a0, a1, F32) for (a0, a1) in A_SL]
        bp_sl = [load("bp", p0, p1, F32) for (p0, p1) in A_SL]
        ba_row = load("ba_row", 0, 1)
        bc_row = load("bc_row", 0, 1)
        bg_row = load("bg_row", 0, 1)
        dmask_sb = [load("dmask", t0, t1) for (t0, t1) in [(0, 128), (128, 256)]]
        bdist = load("bdist", 0, 128, F32)

        # bias2d pair tiles [x-blk, 512] bf16 = bdist * (dist>=10), duplicated
        bias2d = []
        for xb in range(2):
            b2 = C.tile([128, 512], BF16, tag=f"bias2d_{xb}", name=f"bias2d_{xb}")
            for h in range(2):
                nc.vector.tensor_scalar_mul(
                    b2[:, h * 256:(h + 1) * 256], dmask_sb[xb][:], bdist[:, :1])
            bias2d.append(b2)

        P_f = [C.tile([NIT, 400], F32, tag=f"P{s}", name=f"P{s}") for s in range(2)]

        # ---------------- pipeline stages ----------------
        state = {}

        def rden_to_sb(denst, prefix):
            """denst [128,4] f32 cols (h,blk) -> sbuf [128,512] bf16 of
            reciprocal denominators broadcast down partitions."""
            rden = wtile(f"{prefix}_rd", (128, 4), F32, bufs=4)
            nc.vector.reciprocal(rden[:], denst[:])
            rdbf = wtile(f"{prefix}_rdb", (128, 4), BF16, bufs=4)
            nc.vector.tensor_copy(rdbf[:], rden[:])
            rowps = ps_tile([1, 512], BF16)
            for h in range(2):
                for blk in range(2):
                    c = 2 * h + blk
                    nc.tensor.transpose(
                        rowps[:1, h * 256 + blk * 128: h * 256 + (blk + 1) * 128],
                        rdbf[:, c:c + 1], ident[:])
            rrow = wtile(f"{prefix}_rr", (1, 512), BF16, bufs=4)
            nc.scalar.copy(rrow[:], rowps[:1, :])
            bps = ps_tile([128, 512])
            nc.tensor.matmul(bps[:, :], lhsT=ones_bf[:1, :128], rhs=rrow[:1, :],
                             start=True, stop=True)
            rb = wtile(f"{prefix}_rb", (128, 512), BF16, bufs=2)
            nc.vector.tensor_copy(rb[:], bps[:, :])
            return rb

        def stage0(p):
            """norms (squares + Newton rsqrt + scale), eT, fT for both sides."""
            st = state.setdefault(p, {})
            for s in range(2):
                for ti in range(2):
                    for h in range(2):
                        g = gcol(p, s, ti, h)
                        scr = W.tile([128, 300], BF16, tag="sqscr", name="sqscr",
                                     bufs=4)
                        src_ap = e_n[(s, ti, p)][:, h * 300:(h + 1) * 300]
                        if (ti + h) % 2 == 0:
                            nc.scalar.activation(scr[:], src_ap, AF.Square,
                                                 accum_out=invsq[:, g:g + 1])
                        else:
                            nc.vector.scalar_tensor_tensor(
                                scr[:], src_ap, 1.0, src_ap,
                                op0=ALU.mult, op1=ALU.mult,
                                accum_out=invsq[:, g:g + 1])
            # Newton rsqrt on [128, 8]: magic seed + 2 iterations
            c0, c1 = 8 * p, 8 * p + 8
            x = invsq[:, c0:c1]
            it_ = wtile("nwt_i", (128, 8), I32, bufs=2)
            nc.vector.tensor_scalar(it_[:], x.bitcast(I32), 1, None,
                                    op0=ALU.arith_shift_right)
            nc.vector.tensor_scalar(it_[:], it_[:], -1, 0x5F3759DF,
                                    op0=ALU.mult, op1=ALU.add)
            y = it_[:].bitcast(F32)
            t1 = wtile("nwt_t", (128, 8), F32, bufs=2)
            for _ in range(2):
                nc.vector.tensor_mul(t1[:], y, y)
                nc.vector.tensor_mul(t1[:], t1[:], x)
                nc.vector.tensor_scalar(t1[:], t1[:], -0.5, 1.5,
                                        op0=ALU.mult, op1=ALU.add)
                nc.vector.tensor_mul(y, y, t1[:])
            nc.vector.tensor_copy(inv[:, c0:c1], y)
            for s in range(2):
                for ti in range(2):
                    for h in range(2):
                        g = gcol(p, s, ti, h)
                        t = e_n[(s, ti, p)]
                        nc.vector.tensor_scalar_mul(
                            t[:, h * 300:(h + 1) * 300],
                            t[:, h * 300:(h + 1) * 300], inv[:, g:g + 1])
            for s in range(2):
                eT = []
                for di, (d0, d1) in enumerate(D_SL):
                    dsz = d1 - d0
                    tps = ps_tile([128, 512], BF16)
                    for h in range(2):
                        for ti in range(2):
                            nc.tensor.transpose(
                                tps[:dsz, h * 256 + ti * 128:
                                    h * 256 + (ti + 1) * 128],
                                e_n[(s, ti, p)][:, h * 300 + d0: h * 300 + d1],
                                ident[:])
                    t = wtile(f"eT{s}{di}", bufs=3)
                    nc.vector.tensor_copy(t[:dsz, :], tps[:dsz, :])
                    eT.append(t)
                st[f"eT{s}"] = eT
            for s in range(2):
                fT = []
                for ai, (a0, a1) in enumerate(A_SL):
                    asz = a1 - a0
                    ps = ps_tile([128, 512])
                    for k in range(3):
                        ksz = D_SL[k][1] - D_SL[k][0]
                        nc.tensor.matmul(ps[:asz, :],
                                         lhsT=wi_k[k][:ksz, a0:a1],
                                         rhs=st[f"eT{s}"][k][:ksz, :],
                                         start=(k == 0), stop=(k == 2))
                    t = wtile(f"fT{s}{ai}", bufs=2)
                    nc.scalar.activation(t[:asz, :], ps[:asz, :], AF.Relu,
                                         bias=bi_sl[ai][:asz, :1])
                    fT.append(t)
                st[f"fT{s}"] = fT

        def stage1(p):
            """att (+bias via identity matmul), exp, rden, xp for both sides."""
            st = state[p]
            for s in range(2):
                fT = st[f"fT{s}"]
                denst = wtile(f"iden{s}", (128, 4), F32, bufs=3)
                E = []
                att_ps = []
                for xb in range(2):
                    ps = ps_tile([128, 512])
                    for h in range(2):
                        for ai, (a0, a1) in enumerate(A_SL):
                            asz = a1 - a0
                            nc.tensor.matmul(
                                ps[:, h * 256:(h + 1) * 256],
                                lhsT=fT[ai][:asz, h * 256 + xb * 128:
                                            h * 256 + (xb + 1) * 128],
                                rhs=fT[ai][:asz, h * 256:(h + 1) * 256],
                                start=(ai == 0), stop=False)
                        nc.tensor.matmul(
                            ps[:, h * 256:(h + 1) * 256],
                            lhsT=ident[:, :], rhs=bias2d[xb][:, h * 256:(h + 1) * 256],
                            start=False, stop=True)
                    att_ps.append(ps)
                for xb in range(2):
                    et = wtile(f"E{s}{xb}", bufs=2)
                    for h in range(2):
                        nc.scalar.activation(
                            et[:, h * 256:(h + 1) * 256],
                            att_ps[xb][:, h * 256:(h + 1) * 256], AF.Exp,
                            accum_out=denst[:, 2 * h + xb: 2 * h + xb + 1])
                    E.append(et)
                st[f"E{s}"] = E
                st[f"denI{s}"] = denst
            for s in range(2):
                rb = rden_to_sb(st[f"denI{s}"], f"i{s}")
                E = st[f"E{s}"]
                xpT = []
                for di, (d0, d1) in enumerate(D_SL):
                    dsz = d1 - d0
                    ps = ps_tile([128, 512])
                    for h in range(2):
                        for ti in range(2):
                            nc.tensor.matmul(
                                ps[:dsz, h * 256:(h + 1) * 256],
                                lhsT=e_n[(s, ti, p)][:, h * 300 + d0: h * 300 + d1],
                                rhs=E[ti][:, h * 256:(h + 1) * 256],
                                start=(ti == 0), stop=(ti == 1))
                    t = wtile(f"xp{s}{di}", bufs=2)
                    nc.vector.tensor_mul(t[:dsz, :], ps[:dsz, :], rb[:dsz, :])
                    xpT.append(t)
                st[f"xp{s}"] = xpT

        def stage2(p):
            """pT, pRow, aT for both sides."""
            st = state[p]
            for s in range(2):
                hT = st[f"eT{s}"] + st[f"xp{s}"]
                pT = []
                for pi, (p0, p1) in enumerate(A_SL):
                    psz = p1 - p0
                    ps = ps_tile([128, 512])
                    for k in range(6):
                        ksz = D_SL[k % 3][1] - D_SL[k % 3][0]
                        nc.tensor.matmul(ps[:psz, :], lhsT=wp_k[k][:ksz, p0:p1],
                                         rhs=hT[k][:ksz, :], start=(k == 0),
                                         stop=(k == 5))
                    t = wtile(f"pT{s}{pi}", bufs=3)
                    nc.scalar.activation(t[:psz, :], ps[:psz, :], AF.Identity,
                                         bias=bp_sl[pi][:psz, :1])
                    pT.append(t)
                st[f"pT{s}"] = pT
            for s in range(2):
                pT = st[f"pT{s}"]
                pRow = []
                for ti in range(2):
                    tps = ps_tile([128, 400], BF16)
                    for h in range(2):
                        for pi, (p0, p1) in enumerate(A_SL):
                            psz = p1 - p0
                            nc.tensor.transpose(
                                tps[:, h * 200 + p0: h * 200 + p1],
                                pT[pi][:psz, h * 256 + ti * 128:
                                       h * 256 + (ti + 1) * 128],
                                ident[:psz, :psz])
                    t = wtile(f"pR{s}{ti}", (128, 400), bufs=3)
                    nc.scalar.copy(t[:], tps[:, :])
                    pRow.append(t)
                st[f"pR{s}"] = pRow
            maskrow = {}
            for s in range(2):
                rps = ps_tile([1, 512], BF16)
                for h in range(2):
                    it = 2 * p + h
                    for ti in range(2):
                        nc.tensor.transpose(
                            rps[:1, h * 256 + ti * 128: h * 256 + (ti + 1) * 128],
                            mcol_b[s][ti][:, it:it + 1], ident[:])
                mrow = wtile(f"mrow{s}", (1, 512), BF16, bufs=2)
                nc.vector.tensor_copy(mrow[:], rps[:1, :])
                bps = ps_tile([128, 512])
                nc.tensor.matmul(bps[:, :], lhsT=ones_bf[:1, :128],
                                 rhs=mrow[:1, :], start=True, stop=True)
                mr = wtile(f"mrowB{s}", (128, 512), BF16, bufs=2)
                nc.vector.tensor_copy(mr[:], bps[:, :])
                maskrow[s] = mr
            for s in range(2):
                pT = st[f"pT{s}"]
                aT = []
                for ai, (a0, a1) in enumerate(A_SL):
                    asz = a1 - a0
                    ps = ps_tile([128, 512])
                    for ki, (k0, k1) in enumerate(A_SL):
                        ksz = k1 - k0
                        nc.tensor.matmul(ps[:asz, :], lhsT=wa_k[ki][:ksz, a0:a1],
                                         rhs=pT[ki][:ksz, :], start=(ki == 0),
                                         stop=False)
                    nc.tensor.matmul(ps[:asz, :], lhsT=ba_row[:1, a0:a1],
                                     rhs=ones_bf[:1, :], start=False, stop=True)
                    t = wtile(f"aT{s}{ai}", bufs=2)
                    nc.vector.scalar_tensor_tensor(
                        t[:asz, :], ps[:asz, :], 0.0, maskrow[s][:asz, :],
                        op0=ALU.max, op1=ALU.mult)
                    aT.append(t)
                st[f"aT{s}"] = aT

        def stage3(p):
            """sim & simT, exps with mask scale, rden broadcasts."""
            st = state[p]
            a1T, a2T = st["aT0"], st["aT1"]
            den2 = wtile("den2", (128, 4), F32, bufs=3)
            den1 = wtile("den1", (128, 4), F32, bufs=3)
            E2, E1 = [], []
            sim_ps, simT_ps = [], []
            for xb in range(2):
                ps = ps_tile([128, 512])
                for h in range(2):
                    for ai, (a0, a1) in enumerate(A_SL):
                        asz = a1 - a0
                        nc.tensor.matmul(
                            ps[:, h * 256:(h + 1) * 256],
                            lhsT=a1T[ai][:asz, h * 256 + xb * 128:
                                         h * 256 + (xb + 1) * 128],
                            rhs=a2T[ai][:asz, h * 256:(h + 1) * 256],
                            start=(ai == 0), stop=(ai == 1))
                sim_ps.append(ps)
            for xb in range(2):
                et = wtile(f"E2_{xb}", bufs=2)
                for h in range(2):
                    it = 2 * p + h
                    nc.scalar.activation(
                        et[:, h * 256:(h + 1) * 256],
                        sim_ps[xb][:, h * 256:(h + 1) * 256], AF.Exp,
                        scale=mcol_f[0][xb][:, it:it + 1],
                        accum_out=den2[:, 2 * h + xb: 2 * h + xb + 1])
                E2.append(et)
            for yb in range(2):
                ps = ps_tile([128, 512])
                for h in range(2):
                    for ai, (a0, a1) in enumerate(A_SL):
                        asz = a1 - a0
                        nc.tensor.matmul(
                            ps[:, h * 256:(h + 1) * 256],
                            lhsT=a2T[ai][:asz, h * 256 + yb * 128:
                                         h * 256 + (yb + 1) * 128],
                            rhs=a1T[ai][:asz, h * 256:(h + 1) * 256],
                            start=(ai == 0), stop=(ai == 1))
                simT_ps.append(ps)
            for yb in range(2):
                et = wtile(f"E1_{yb}", bufs=2)
                for h in range(2):
                    it = 2 * p + h
                    nc.scalar.activation(
                        et[:, h * 256:(h + 1) * 256],
                        simT_ps[yb][:, h * 256:(h + 1) * 256], AF.Exp,
                        scale=mcol_f[1][yb][:, it:it + 1],
                        accum_out=den1[:, 2 * h + yb: 2 * h + yb + 1])
                E1.append(et)
            st["E2"], st["E1"] = E2, E1
            st["b2"] = rden_to_sb(den2, "x2")
            st["b1"] = rden_to_sb(den1, "x1")

        def stage4(p):
            """betaT / alphaT with drain-time normalization."""
            st = state[p]
            betaT, alphaT = [], []
            for pi, (p0, p1) in enumerate(A_SL):
                psz = p1 - p0
                ps = ps_tile([128, 512])
                for h in range(2):
                    for ti in range(2):
                        nc.tensor.matmul(
                            ps[:psz, h * 256:(h + 1) * 256],
                            lhsT=st["pR1"][ti][:, h * 200 + p0: h * 200 + p1],
                            rhs=st["E1"][ti][:, h * 256:(h + 1) * 256],
                            start=(ti == 0), stop=(ti == 1))
                t = wtile(f"bT{pi}", bufs=2)
                nc.vector.tensor_mul(t[:psz, :], ps[:psz, :], st["b2"][:psz, :])
                betaT.append(t)
            for pi, (p0, p1) in enumerate(A_SL):
                psz = p1 - p0
                ps = ps_tile([128, 512])
                for h in range(2):
                    for xb in range(2):
                        nc.tensor.matmul(
                            ps[:psz, h * 256:(h + 1) * 256],
                            lhsT=st["pR0"][xb][:, h * 200 + p0: h * 200 + p1],
                            rhs=st["E2"][xb][:, h * 256:(h + 1) * 256],
                            start=(xb == 0), stop=(xb == 1))
                t = wtile(f"alT{pi}", bufs=2)
                nc.vector.tensor_mul(t[:psz, :], ps[:psz, :], st["b1"][:psz, :])
                alphaT.append(t)
            st["betaT"], st["alphaT"] = betaT, alphaT

        def stage5(p):
            """compare + bias + relu + PE pooling + row stash."""
            st = state[p]
            for s, pTt, oT in ((0, st["pT0"], st["betaT"]),
                               (1, st["pT1"], st["alphaT"])):
                kt = pTt + oT
                for h in range(2):
                    it = 2 * p + h
                    vrs = []
                    for ti in range(2):
                        cps = ps_tile([128, 400])
                        for k in range(4):
                            ksz = WC_K[k][1] - WC_K[k][0]
                            nc.tensor.matmul(
                                cps[:, :],
                                lhsT=kt[k][:ksz, h * 256 + ti * 128:
                                           h * 256 + (ti + 1) * 128],
                                rhs=wc_k[k][:ksz, :400],
                                start=(k == 0), stop=False)
                        nc.tensor.matmul(cps[:, :], lhsT=ones_bf[:1, :128],
                                         rhs=bc_row[:1, :400],
                                         start=False, stop=True)
                        vr = wtile("vr", (128, 400), BF16, bufs=3)
                        if ti == 0:
                            nc.vector.tensor_scalar(vr[:], cps[:, :], 0.0, None,
                                                    op0=ALU.max)
                        else:
                            nc.scalar.activation(vr[:], cps[:, :], AF.Relu)
                        vrs.append(vr)
                    pps = ps_tile([1, 400])
                    for ti in range(2):
                        nc.tensor.matmul(pps[:1, :],
                                         lhsT=mcol_b[s][ti][:, it:it + 1],
                                         rhs=vrs[ti][:, :],
                                         start=(ti == 0), stop=(ti == 1))
                    prow = wtile("prow", (1, 400), F32, bufs=3)
                    if s == 0:
                        nc.scalar.copy(prow[:], pps[:1, :])
                    else:
                        nc.vector.tensor_copy(prow[:], pps[:1, :])
                    nc.sync.dma_start(out=P_f[s][it:it + 1, :], in_=prow[:1, :])
            del state[p]

        stages = [stage0, stage1, stage2, stage3, stage4, stage5]
        NST = len(stages)
        for t in range(NPAIR + NST - 1):
            for k in reversed(range(NST)):
                p = t - k
                if 0 <= p < NPAIR:
                    stages[k](p)

        # ---------------- aggregate ----------------
        PT_sb = []
        for s in range(2):
            pb = C.tile([NIT, 400], BF16, tag=f"Pb{s}", name=f"Pb{s}")
            nc.vector.tensor_copy(pb[:], P_f[s][:])
            for c, (c0, c1) in enumerate(V_CH):
                csz = c1 - c0
                tps = ps_tile([128, NIT], BF16)
                nc.tensor.transpose(tps[:csz, :NIT], pb[:NIT, c0:c1],
                                    ident[:NIT, :NIT])
                t = C.tile([128, NIT], BF16, tag=f"PT{s}_{c}", name=f"PT{s}_{c}")
                nc.scalar.copy(t[:csz, :], tps[:csz, :])
                PT_sb.append(t)
        aps = ps_tile([CLS, NIT])
        for k in range(8):
            ksz = V_CH[k % 4][1] - V_CH[k % 4][0]
            nc.tensor.matmul(aps[:, :], lhsT=wg_k[k][:ksz, :CLS],
                             rhs=PT_sb[k][:ksz, :], start=(k == 0), stop=False)
        nc.tensor.matmul(aps[:, :], lhsT=bg_row[:1, :CLS],
                         rhs=ones_bf[:1, :NIT], start=False, stop=True)
        out_sb = C.tile([CLS, NIT], F32)
        nc.scalar.copy(out_sb[:], aps[:, :])
        nc.sync.dma_start(out=out_d.ap(), in_=out_sb[:])


def _get_nc():
    global _CACHED_NC
    if _CACHED_NC is None:
        _CACHED_NC = _build_nc()
    return _CACHED_NC


def make_in_maps(inputs):
    x1 = np.asarray(inputs["x1"])
    x2 = np.asarray(inputs["x2"])
    f32 = lambda k: np.ascontiguousarray(np.asarray(inputs[k], dtype=np.float32))
    bf = lambda a: np.ascontiguousarray(np.asarray(a, dtype=np.float32)).astype(BF_NP)
    ii, jj = np.meshgrid(np.arange(L), np.arange(L), indexing="ij")
    dmask = (np.abs(ii - jj) >= 10).astype(np.float32)
    bdist = np.full((128, 1), np.asarray(inputs["b_dist"], np.float32).reshape(-1)[0],
                    np.float32)

    shared = {
        "emb": bf(inputs["emb"]),
        "wi": bf(inputs["Wi"]), "wp": bf(inputs["Wp"]), "wa": bf(inputs["Wa"]),
        "wc": bf(inputs["Wc"]), "wg": bf(inputs["Wg"]),
        "bi": f32("bi").reshape(-1, 1), "bp": f32("bp").reshape(-1, 1),
        "ba_row": bf(np.asarray(inputs["ba"]).reshape(1, -1)),
        "bc_row": bf(np.asarray(inputs["bc"]).reshape(1, -1)),
        "bg_row": bf(np.asarray(inputs["bg"]).reshape(1, -1)),
        "dmask": dmask.astype(BF_NP), "bdist": bdist,
    }
    in_maps = []
    for c in range(NCORES):
        sl = slice(c * NIT, (c + 1) * NIT)
        x1s = np.ascontiguousarray(x1[sl]).astype(np.int32)
        x2s = np.ascontiguousarray(x2[sl]).astype(np.int32)
        m = dict(shared)
        m["idx1"] = np.ascontiguousarray(x1s.reshape(-1).reshape(2 * NIT, 128).T)
        m["idx2"] = np.ascontiguousarray(x2s.reshape(-1).reshape(2 * NIT, 128).T)
        m["xi1"] = x1s
        m["xi2"] = x2s
        in_maps.append(m)
    return in_maps


def kernel(**inputs):
    nc = _get_nc()
    in_maps = make_in_maps(inputs)
    res = run_bass_kernel_spmd(nc, in_maps, core_ids=list(range(NCORES)))
    out = np.concatenate([r["out"].T for r in res.results], axis=0)
    return np.ascontiguousarray(out, dtype=np.float32)


# revision 15
# speedup vs baseline: 2.2017x; 1.3747x over previous
"""Trainium2 Bass kernel for nn_Decomposeable (decomposable attention model).

Strategy: data-parallel over batch B=128 across 8 NeuronCores (16 items/core,
processed as 8 pairs with free-dim-512 matmuls for all shared-weight FCs).
All matmul operands bf16 (fp32 PSUM accumulate). Softmax is transpose-free:
the intra attention matrix is symmetric and the cross attention is computed
in both orientations by direct matmuls; attention-weight matmuls consume the
raw exp tiles and the reciprocal softmax denominators are applied at PSUM
drain time via a ones-outer-product broadcast. Sequence masks fold into the
exp scale column and the attended operand rows; pooling runs on the PE with
the mask column as lhsT. Per-pair work is emitted as a 6-stage software
pipeline so every cross-engine latency is covered by other pairs' matmuls.
"""
import sys
import numpy as np

for _p in ("/opt/trn_rl_repo",):
    if _p not in sys.path:
        sys.path.append(_p)

import ml_dtypes
import concourse.bass as bass
import concourse.bacc as bacc
import concourse.tile as tile
from concourse import mybir
from concourse.bass_utils import run_bass_kernel_spmd
from concourse.masks import make_identity

F32 = mybir.dt.float32
BF16 = mybir.dt.bfloat16
I32 = mybir.dt.int32
AF = mybir.ActivationFunctionType
ALU = mybir.AluOpType
AX = mybir.AxisListType
BF_NP = ml_dtypes.bfloat16

L, EMB, PROJ, ATT, CLS = 256, 300, 200, 200, 3
B, NCORES = 128, 8
NIT = B // NCORES            # items per core
NPAIR = NIT // 2
VOCAB = 50000

D_SL = [(0, 128), (128, 256), (256, 300)]          # EMB k-tiles
A_SL = [(0, 128), (128, 200)]                      # ATT/PROJ tiles
WC_K = [(0, 128), (128, 200), (200, 328), (328, 400)]
V_CH = [(0, 128), (128, 256), (256, 384), (384, 400)]  # P transpose chunks

_CACHED_NC = None


def _build_nc():
    nc = bacc.Bacc("TRN2", target_bir_lowering=False, debug=False)

    dram = {}
    def din(name, shape, dt):
        dram[name] = nc.dram_tensor(name, shape, dt, kind="ExternalInput")
        return dram[name]

    din("idx1", [128, 2 * NIT], I32)
    din("idx2", [128, 2 * NIT], I32)
    din("xi1", [NIT, L], I32)
    din("xi2", [NIT, L], I32)
    din("emb", [VOCAB, EMB], BF16)
    din("wi", [EMB, ATT], BF16)
    din("wp", [2 * EMB, PROJ], BF16)
    din("wa", [PROJ, ATT], BF16)
    din("wc", [2 * PROJ, 2 * PROJ], BF16)
    din("wg", [4 * PROJ, CLS], BF16)
    din("bi", [ATT, 1], F32)
    din("bp", [PROJ, 1], F32)
    din("ba_row", [1, ATT], BF16)
    din("bc_row", [1, 2 * PROJ], BF16)
    din("bg_row", [1, CLS], BF16)
    din("dmask", [L, L], BF16)
    din("bdist", [128, 1], F32)
    out_d = nc.dram_tensor("out", [CLS, NIT], F32, kind="ExternalOutput")

    with tile.TileContext(nc) as tc:
        _emit(nc, tc, dram, out_d)
    nc.compile()
    return nc


def _emit(nc, tc, dram, out_d):
    from contextlib import ExitStack
    ctx = ExitStack()
    with ctx:
        C = ctx.enter_context(tc.tile_pool(name="consts", bufs=1))
        PS = ctx.enter_context(tc.tile_pool(name="ps", bufs=7, space="PSUM"))
        W = ctx.enter_context(tc.tile_pool(name="work", bufs=3))

        def ps_tile(shape, dt=F32):
            return PS.tile(shape, dt, tag="ps", name="ps")

        def wtile(tag, shape=(128, 512), dt=BF16, bufs=3):
            return W.tile(list(shape), dt, tag=tag, name=tag, bufs=bufs)

        # ---------------- small input DMAs first ----------------
        idx_sb = {}
        for s, name in ((0, "idx1"), (1, "idx2")):
            t = C.tile([128, 2 * NIT], I32, tag=name, name=name)
            nc.sync.dma_start(out=t[:], in_=dram[name].ap())
            idx_sb[s] = t
        xi_sb = {}
        for s, name in ((0, "xi1"), (1, "xi2")):
            t = C.tile([NIT, L], I32, tag=name, name=name)
            nc.sync.dma_start(out=t[:], in_=dram[name].ap())
            xi_sb[s] = t

        dmask_d = dram["dmask"].ap()
        dmask_sb = []
        for (t0, t1) in [(0, 128), (128, 256)]:
            t = C.tile([128, 256], BF16, tag=f"dmask_{t0}", name=f"dmask_{t0}")
            nc.sync.dma_start(out=t[:t1 - t0, :], in_=dmask_d[t0:t1, :])
            dmask_sb.append(t)
        bdist = C.tile([128, 1], F32, tag="bdist", name="bdist")
        nc.sync.dma_start(out=bdist[:], in_=dram["bdist"].ap())

        # ---------------- constants ----------------
        ident_f = C.tile([128, 128], F32)
        make_identity(nc, ident_f[:])
        ident = C.tile([128, 128], BF16)
        nc.vector.tensor_copy(ident[:], ident_f[:])
        ones_bf = C.tile([1, 512], BF16)
        nc.vector.memset(ones_bf[:], 1.0)
        iota_i = C.tile([NIT, L], I32)
        nc.gpsimd.iota(iota_i[:], pattern=[[1, L]], base=0, channel_multiplier=0)
        iota16 = C.tile([NIT, L], F32)
        nc.vector.tensor_copy(iota16[:], iota_i[:])

        # ---------------- embedding gathers + squares (gpsimd) ----------------
        e_n = {}
        invsq = C.tile([128, 8 * NPAIR], F32)
        inv = C.tile([128, 8 * NPAIR], F32)

        def gcol(p, s, ti, h):
            return 8 * p + 4 * s + 2 * ti + h

        for p in range(NPAIR):
            for s in range(2):
                for ti in range(2):
                    t = C.tile([128, 600], BF16, tag=f"e_{s}_{ti}_{p}",
                               name=f"e_{s}_{ti}_{p}")
                    e_n[(s, ti, p)] = t
                    for h in range(2):
                        g = 2 * (2 * p + h) + ti
                        nc.gpsimd.indirect_dma_start(
                            out=t[:, h * 300:(h + 1) * 300], out_offset=None,
                            in_=dram["emb"].ap(),
                            in_offset=bass.IndirectOffsetOnAxis(
                                ap=idx_sb[s][:, g:g + 1], axis=0))

        # ---------------- masks ----------------
        m_bf = {}
        mcol_f = {}
        mcol_b = {}
        for s in range(2):
            xf = W.tile([NIT, L], F32, tag="xf", name="xf", bufs=1)
            nc.vector.tensor_copy(xf[:], xi_sb[s][:])
            nz = W.tile([NIT, L], F32, tag="nz", name="nz", bufs=1)
            nc.vector.tensor_scalar(nz[:], xf[:], 0.0, None, op0=ALU.not_equal)
            sizes = C.tile([NIT, 1], F32, tag=f"sizes{s}", name=f"sizes{s}")
            nc.vector.tensor_reduce(sizes[:], nz[:], axis=AX.X, op=ALU.add)
            mb = C.tile([NIT, L], BF16, tag=f"mbf{s}", name=f"mbf{s}")
            nc.vector.tensor_scalar(mb[:], iota16[:], sizes[:, :1], None,
                                    op0=ALU.is_lt)
            m_bf[s] = mb
            cf, cb = [], []
            for ti in range(2):
                pt = ps_tile([128, NIT], BF16)
                nc.tensor.transpose(pt[:, :NIT], mb[:NIT, ti * 128:(ti + 1) * 128],
                                    ident[:NIT, :NIT])
                f = C.tile([128, NIT], F32, tag=f"mcf{s}_{ti}", name=f"mcf{s}_{ti}")
                nc.vector.tensor_copy(f[:], pt[:, :NIT])
                bt = C.tile([128, NIT], BF16, tag=f"mcb{s}_{ti}", name=f"mcb{s}_{ti}")
                nc.scalar.copy(bt[:], pt[:, :NIT])
                cf.append(f)
                cb.append(bt)
            mcol_f[s] = cf
            mcol_b[s] = cb

        # ---------------- weights ----------------
        def load(name, r0, r1, dt=BF16):
            src = dram[name].ap()
            w = src.shape[1]
            t = C.tile([128, w], dt, tag=f"{name}_{r0}", name=f"{name}_{r0}")
            nc.sync.dma_start(out=t[:r1 - r0, :], in_=src[r0:r1, :])
            return t

        wi_k = [load("wi", d0, d1) for (d0, d1) in D_SL]
        wp_k = [load("wp", d0, d1) for (d0, d1) in D_SL] + \
               [load("wp", 300 + d0, 300 + d1) for (d0, d1) in D_SL]
        wa_k = [load("wa", a0, a1) for (a0, a1) in A_SL]
        wc_k = [load("wc", k0, k1) for (k0, k1) in WC_K]
        wg_k = [load("wg", s * 400 + v0, s * 400 + v1)
                for s in range(2) for (v0, v1) in V_CH]
        bi_sl = [load("bi", a0, a1, F32) for (a0, a1) in A_SL]
        bp_sl = [load("bp", p0, p1, F32) for (p0, p1) in A_SL]
        ba_row = load("ba_row", 0, 1)
        bc_row = load("bc_row", 0, 1)
        bg_row = load("bg_row", 0, 1)

        # bias2d pair tiles [x-blk, 512] bf16 = bdist * (dist>=10), duplicated
        bias2d = []
        for xb in range(2):
            b2 = C.tile([128, 512], BF16, tag=f"bias2d_{xb}", name=f"bias2d_{xb}")
            for h in range(2):
                nc.vector.tensor_scalar_mul(
                    b2[:, h * 256:(h + 1) * 256], dmask_sb[xb][:], bdist[:, :1])
            bias2d.append(b2)

        P_f = [C.tile([NIT, 400], F32, tag=f"P{s}", name=f"P{s}") for s in range(2)]

        # ---------------- pipeline stages ----------------
        state = {}

        def rden_pre(denst, prefix):
            """denst [128,4] f32 cols (h,blk) -> rrow [1,512] bf16 of
            reciprocal denominators (via PE column->row transposes)."""
            rden = wtile(f"{prefix}_rd", (128, 4), F32, bufs=4)
            nc.vector.reciprocal(rden[:], denst[:])
            rdbf = wtile(f"{prefix}_rdb", (128, 4), BF16, bufs=4)
            nc.vector.tensor_copy(rdbf[:], rden[:])
            rowps = ps_tile([1, 512], BF16)
            for h in range(2):
                for blk in range(2):
                    c = 2 * h + blk
                    nc.tensor.transpose(
                        rowps[:1, h * 256 + blk * 128: h * 256 + (blk + 1) * 128],
                        rdbf[:, c:c + 1], ident[:])
            rrow = wtile(f"{prefix}_rr", (1, 512), BF16, bufs=4)
            nc.scalar.copy(rrow[:], rowps[:1, :])
            return rrow

        def rden_bcast(rrow, prefix):
            bps = ps_tile([128, 512])
            nc.tensor.matmul(bps[:, :], lhsT=ones_bf[:1, :128], rhs=rrow[:1, :],
                             start=True, stop=True)
            rb = wtile(f"{prefix}_rb", (128, 512), BF16, bufs=2)
            nc.vector.tensor_copy(rb[:], bps[:, :])
            return rb

        def stage0(p):
            """norms (squares + Newton rsqrt + scale), eT, fT for both sides."""
            st = state.setdefault(p, {})
            for s in range(2):
                for ti in range(2):
                    for h in range(2):
                        g = gcol(p, s, ti, h)
                        scr = W.tile([128, 300], BF16, tag="sqscr", name="sqscr",
                                     bufs=4)
                        src_ap = e_n[(s, ti, p)][:, h * 300:(h + 1) * 300]
                        if (ti + h) % 2 == 0:
                            nc.scalar.activation(scr[:], src_ap, AF.Square,
                                                 accum_out=invsq[:, g:g + 1])
                        else:
                            nc.vector.scalar_tensor_tensor(
                                scr[:], src_ap, 1.0, src_ap,
                                op0=ALU.mult, op1=ALU.mult,
                                accum_out=invsq[:, g:g + 1])
            # Newton rsqrt on [128, 8]: magic seed + 2 iterations
            c0, c1 = 8 * p, 8 * p + 8
            x = invsq[:, c0:c1]
            it_ = wtile("nwt_i", (128, 8), I32, bufs=2)
            nc.vector.tensor_scalar(it_[:], x.bitcast(I32), 1, None,
                                    op0=ALU.arith_shift_right)
            nc.vector.tensor_scalar(it_[:], it_[:], -1, 0x5F3759DF,
                                    op0=ALU.mult, op1=ALU.add)
            y = it_[:].bitcast(F32)
            t1 = wtile("nwt_t", (128, 8), F32, bufs=2)
            for _ in range(2):
                nc.vector.tensor_mul(t1[:], y, y)
                nc.vector.tensor_mul(t1[:], t1[:], x)
                nc.vector.tensor_scalar(t1[:], t1[:], -0.5, 1.5,
                                        op0=ALU.mult, op1=ALU.add)
                nc.vector.tensor_mul(y, y, t1[:])
            nc.vector.tensor_copy(inv[:, c0:c1], y)
            for s in range(2):
                for ti in range(2):
                    for h in range(2):
                        g = gcol(p, s, ti, h)
                        t = e_n[(s, ti, p)]
                        nc.vector.tensor_scalar_mul(
                            t[:, h * 300:(h + 1) * 300],
                            t[:, h * 300:(h + 1) * 300], inv[:, g:g + 1])
            for s in range(2):
                eT = []
                for di, (d0, d1) in enumerate(D_SL):
                    dsz = d1 - d0
                    tps = ps_tile([128, 512], BF16)
                    for h in range(2):
                        for ti in range(2):
                            nc.tensor.transpose(
                                tps[:dsz, h * 256 + ti * 128:
                                    h * 256 + (ti + 1) * 128],
                                e_n[(s, ti, p)][:, h * 300 + d0: h * 300 + d1],
                                ident[:])
                    t = wtile(f"eT{s}{di}", bufs=3)
                    nc.vector.tensor_copy(t[:dsz, :], tps[:dsz, :])
                    eT.append(t)
                st[f"eT{s}"] = eT
            for s in range(2):
                fT = []
                for ai, (a0, a1) in enumerate(A_SL):
                    asz = a1 - a0
                    ps = ps_tile([128, 512])
                    for k in range(3):
                        ksz = D_SL[k][1] - D_SL[k][0]
                        nc.tensor.matmul(ps[:asz, :],
                                         lhsT=wi_k[k][:ksz, a0:a1],
                                         rhs=st[f"eT{s}"][k][:ksz, :],
                                         start=(k == 0), stop=(k == 2))
                    t = wtile(f"fT{s}{ai}", bufs=2)
                    nc.scalar.activation(t[:asz, :], ps[:asz, :], AF.Relu,
                                         bias=bi_sl[ai][:asz, :1])
                    fT.append(t)
                st[f"fT{s}"] = fT

        def stage1a(p):
            """att (+bias via identity matmul) and exps for both sides."""
            st = state[p]
            for s in range(2):
                fT = st[f"fT{s}"]
                denst = wtile(f"iden{s}", (128, 4), F32, bufs=3)
                E = []
                att_ps = []
                for xb in range(2):
                    ps = ps_tile([128, 512])
                    for h in range(2):
                        for ai, (a0, a1) in enumerate(A_SL):
                            asz = a1 - a0
                            nc.tensor.matmul(
                                ps[:, h * 256:(h + 1) * 256],
                                lhsT=fT[ai][:asz, h * 256 + xb * 128:
                                            h * 256 + (xb + 1) * 128],
                                rhs=fT[ai][:asz, h * 256:(h + 1) * 256],
                                start=(ai == 0), stop=False)
                        nc.tensor.matmul(
                            ps[:, h * 256:(h + 1) * 256],
                            lhsT=ident[:, :], rhs=bias2d[xb][:, h * 256:(h + 1) * 256],
                            start=False, stop=True)
                    att_ps.append(ps)
                for xb in range(2):
                    et = wtile(f"E{s}{xb}", bufs=2)
                    for h in range(2):
                        nc.scalar.activation(
                            et[:, h * 256:(h + 1) * 256],
                            att_ps[xb][:, h * 256:(h + 1) * 256], AF.Exp,
                            accum_out=denst[:, 2 * h + xb: 2 * h + xb + 1])
                    E.append(et)
                st[f"E{s}"] = E
                st[f"denI{s}"] = denst

        def stage1b(p):
            """rden prep, then per side: xp matmuls, broadcast, drains."""
            st = state[p]
            rrows = [rden_pre(st[f"denI{s}"], f"i{s}") for s in range(2)]
            for s in range(2):
                E = st[f"E{s}"]
                xp_ps = []
                for di, (d0, d1) in enumerate(D_SL):
                    dsz = d1 - d0
                    ps = ps_tile([128, 512])
                    for h in range(2):
                        for ti in range(2):
                            nc.tensor.matmul(
                                ps[:dsz, h * 256:(h + 1) * 256],
                                lhsT=e_n[(s, ti, p)][:, h * 300 + d0: h * 300 + d1],
                                rhs=E[ti][:, h * 256:(h + 1) * 256],
                                start=(ti == 0), stop=(ti == 1))
                    xp_ps.append(ps)
                rb = rden_bcast(rrows[s], f"i{s}")
                xpT = []
                for di, (d0, d1) in enumerate(D_SL):
                    dsz = d1 - d0
                    t = wtile(f"xp{s}{di}", bufs=2)
                    nc.vector.tensor_mul(t[:dsz, :], xp_ps[di][:dsz, :],
                                         rb[:dsz, :])
                    xpT.append(t)
                st[f"xp{s}"] = xpT

        def stage2(p):
            """pT, pRow, aT for both sides."""
            st = state[p]
            for s in range(2):
                hT = st[f"eT{s}"] + st[f"xp{s}"]
                pT = []
                for pi, (p0, p1) in enumerate(A_SL):
                    psz = p1 - p0
                    ps = ps_tile([128, 512])
                    for k in range(6):
                        ksz = D_SL[k % 3][1] - D_SL[k % 3][0]
                        nc.tensor.matmul(ps[:psz, :], lhsT=wp_k[k][:ksz, p0:p1],
                                         rhs=hT[k][:ksz, :], start=(k == 0),
                                         stop=(k == 5))
                    t = wtile(f"pT{s}{pi}", bufs=3)
                    nc.scalar.activation(t[:psz, :], ps[:psz, :], AF.Identity,
                                         bias=bp_sl[pi][:psz, :1])
                    pT.append(t)
                st[f"pT{s}"] = pT
            for s in range(2):
                pT = st[f"pT{s}"]
                pRow = []
                for ti in range(2):
                    tps = ps_tile([128, 400], BF16)
                    for h in range(2):
                        for pi, (p0, p1) in enumerate(A_SL):
                            psz = p1 - p0
                            nc.tensor.transpose(
                                tps[:, h * 200 + p0: h * 200 + p1],
                                pT[pi][:psz, h * 256 + ti * 128:
                                       h * 256 + (ti + 1) * 128],
                                ident[:psz, :psz])
                    t = wtile(f"pR{s}{ti}", (128, 400), bufs=3)
                    nc.scalar.copy(t[:], tps[:, :])
                    pRow.append(t)
                st[f"pR{s}"] = pRow
            maskrow = {}
            for s in range(2):
                rps = ps_tile([1, 512], BF16)
                for h in range(2):
                    it = 2 * p + h
                    for ti in range(2):
                        nc.tensor.transpose(
                            rps[:1, h * 256 + ti * 128: h * 256 + (ti + 1) * 128],
                            mcol_b[s][ti][:, it:it + 1], ident[:])
                mrow = wtile(f"mrow{s}", (1, 512), BF16, bufs=2)
                nc.vector.tensor_copy(mrow[:], rps[:1, :])
                bps = ps_tile([128, 512])
                nc.tensor.matmul(bps[:, :], lhsT=ones_bf[:1, :128],
                                 rhs=mrow[:1, :], start=True, stop=True)
                mr = wtile(f"mrowB{s}", (128, 512), BF16, bufs=2)
                nc.vector.tensor_copy(mr[:], bps[:, :])
                maskrow[s] = mr
            for s in range(2):
                pT = st[f"pT{s}"]
                aT = []
                for ai, (a0, a1) in enumerate(A_SL):
                    asz = a1 - a0
                    ps = ps_tile([128, 512])
                    for ki, (k0, k1) in enumerate(A_SL):
                        ksz = k1 - k0
                        nc.tensor.matmul(ps[:asz, :], lhsT=wa_k[ki][:ksz, a0:a1],
                                         rhs=pT[ki][:ksz, :], start=(ki == 0),
                                         stop=False)
                    nc.tensor.matmul(ps[:asz, :], lhsT=ba_row[:1, a0:a1],
                                     rhs=ones_bf[:1, :], start=False, stop=True)
                    t = wtile(f"aT{s}{ai}", bufs=2)
                    nc.vector.scalar_tensor_tensor(
                        t[:asz, :], ps[:asz, :], 0.0, maskrow[s][:asz, :],
                        op0=ALU.max, op1=ALU.mult)
                    aT.append(t)
                st[f"aT{s}"] = aT

        def stage3a(p):
            """sim & simT matmuls and exps with mask scale."""
            st = state[p]
            a1T, a2T = st["aT0"], st["aT1"]
            den2 = wtile("den2", (128, 4), F32, bufs=3)
            den1 = wtile("den1", (128, 4), F32, bufs=3)
            E2, E1 = [], []
            sim_ps, simT_ps = [], []
            for xb in range(2):
                ps = ps_tile([128, 512])
                for h in range(2):
                    for ai, (a0, a1) in enumerate(A_SL):
                        asz = a1 - a0
                        nc.tensor.matmul(
                            ps[:, h * 256:(h + 1) * 256],
                            lhsT=a1T[ai][:asz, h * 256 + xb * 128:
                                         h * 256 + (xb + 1) * 128],
                            rhs=a2T[ai][:asz, h * 256:(h + 1) * 256],
                            start=(ai == 0), stop=(ai == 1))
                sim_ps.append(ps)
            for xb in range(2):
                et = wtile(f"E2_{xb}", bufs=2)
                for h in range(2):
                    it = 2 * p + h
                    nc.scalar.activation(
                        et[:, h * 256:(h + 1) * 256],
                        sim_ps[xb][:, h * 256:(h + 1) * 256], AF.Exp,
                        scale=mcol_f[0][xb][:, it:it + 1],
                        accum_out=den2[:, 2 * h + xb: 2 * h + xb + 1])
                E2.append(et)
            for yb in range(2):
                ps = ps_tile([128, 512])
                for h in range(2):
                    for ai, (a0, a1) in enumerate(A_SL):
                        asz = a1 - a0
                        nc.tensor.matmul(
                            ps[:, h * 256:(h + 1) * 256],
                            lhsT=a2T[ai][:asz, h * 256 + yb * 128:
                                         h * 256 + (yb + 1) * 128],
                            rhs=a1T[ai][:asz, h * 256:(h + 1) * 256],
                            start=(ai == 0), stop=(ai == 1))
                simT_ps.append(ps)
            for yb in range(2):
                et = wtile(f"E1_{yb}", bufs=2)
                for h in range(2):
                    it = 2 * p + h
                    nc.scalar.activation(
                        et[:, h * 256:(h + 1) * 256],
                        simT_ps[yb][:, h * 256:(h + 1) * 256], AF.Exp,
                        scale=mcol_f[1][yb][:, it:it + 1],
                        accum_out=den1[:, 2 * h + yb: 2 * h + yb + 1])
                E1.append(et)
            st["E2"], st["E1"] = E2, E1
            st["den2"], st["den1"] = den2, den1

        def stage3b(p):
            """betaT / alphaT matmuls with drain-time normalization."""
            st = state[p]
            rr2 = rden_pre(st["den2"], "x2")
            betaT, alphaT = [], []
            beta_ps = []
            for pi, (p0, p1) in enumerate(A_SL):
                psz = p1 - p0
                ps = ps_tile([128, 512])
                for h in range(2):
                    for ti in range(2):
                        nc.tensor.matmul(
                            ps[:psz, h * 256:(h + 1) * 256],
                            lhsT=st["pR1"][ti][:, h * 200 + p0: h * 200 + p1],
                            rhs=st["E1"][ti][:, h * 256:(h + 1) * 256],
                            start=(ti == 0), stop=(ti == 1))
                beta_ps.append(ps)
            b2 = rden_bcast(rr2, "x2")
            for pi, (p0, p1) in enumerate(A_SL):
                psz = p1 - p0
                t = wtile(f"bT{pi}", bufs=2)
                nc.vector.tensor_mul(t[:psz, :], beta_ps[pi][:psz, :],
                                     b2[:psz, :])
                betaT.append(t)
            rr1 = rden_pre(st["den1"], "x1")
            alpha_ps = []
            for pi, (p0, p1) in enumerate(A_SL):
                psz = p1 - p0
                ps = ps_tile([128, 512])
                for h in range(2):
                    for xb in range(2):
                        nc.tensor.matmul(
                            ps[:psz, h * 256:(h + 1) * 256],
                            lhsT=st["pR0"][xb][:, h * 200 + p0: h * 200 + p1],
                            rhs=st["E2"][xb][:, h * 256:(h + 1) * 256],
                            start=(xb == 0), stop=(xb == 1))
                alpha_ps.append(ps)
            b1 = rden_bcast(rr1, "x1")
            for pi, (p0, p1) in enumerate(A_SL):
                psz = p1 - p0
                t = wtile(f"alT{pi}", bufs=2)
                nc.vector.tensor_mul(t[:psz, :], alpha_ps[pi][:psz, :],
                                     b1[:psz, :])
                alphaT.append(t)
            st["betaT"], st["alphaT"] = betaT, alphaT

        def stage5(p):
            """compare + bias + relu + PE pooling + row stash."""
            st = state[p]
            for s, pTt, oT in ((0, st["pT0"], st["betaT"]),
                               (1, st["pT1"], st["alphaT"])):
                kt = pTt + oT
                for h in range(2):
                    it = 2 * p + h
                    vrs = []
                    for ti in range(2):
                        cps = ps_tile([128, 400])
                        for k in range(4):
                            ksz = WC_K[k][1] - WC_K[k][0]
                            nc.tensor.matmul(
                                cps[:, :],
                                lhsT=kt[k][:ksz, h * 256 + ti * 128:
                                           h * 256 + (ti + 1) * 128],
                                rhs=wc_k[k][:ksz, :400],
                                start=(k == 0), stop=False)
                        nc.tensor.matmul(cps[:, :], lhsT=ones_bf[:1, :128],
                                         rhs=bc_row[:1, :400],
                                         start=False, stop=True)
                        vr = wtile("vr", (128, 400), BF16, bufs=3)
                        if ti == 0:
                            nc.vector.tensor_scalar(vr[:], cps[:, :], 0.0, None,
                                                    op0=ALU.max)
                        else:
                            nc.scalar.activation(vr[:], cps[:, :], AF.Relu)
                        vrs.append(vr)
                    pps = ps_tile([1, 400])
                    for ti in range(2):
                        nc.tensor.matmul(pps[:1, :],
                                         lhsT=mcol_b[s][ti][:, it:it + 1],
                                         rhs=vrs[ti][:, :],
                                         start=(ti == 0), stop=(ti == 1))
                    prow = wtile("prow", (1, 400), F32, bufs=3)
                    if s == 0:
                        nc.scalar.copy(prow[:], pps[:1, :])
                    else:
                        nc.vector.tensor_copy(prow[:], pps[:1, :])
                    nc.sync.dma_start(out=P_f[s][it:it + 1, :], in_=prow[:1, :])
            del state[p]

        stages = [stage0, stage1a, stage1b, stage2, stage3a, stage3b, stage5]
        NST = len(stages)
        for t in range(NPAIR + NST - 1):
            for k in reversed(range(NST)):
                p = t - k
                if 0 <= p < NPAIR:
                    stages[k](p)

        # ---------------- aggregate ----------------
        PT_sb = []
        for s in range(2):
            pb = C.tile([NIT, 400], BF16, tag=f"Pb{s}", name=f"Pb{s}")
            nc.vector.tensor_copy(pb[:], P_f[s][:])
            for c, (c0, c1) in enumerate(V_CH):
                csz = c1 - c0
                tps = ps_tile([128, NIT], BF16)
                nc.tensor.transpose(tps[:csz, :NIT], pb[:NIT, c0:c1],
                                    ident[:NIT, :NIT])
                t = C.tile([128, NIT], BF16, tag=f"PT{s}_{c}", name=f"PT{s}_{c}")
                nc.scalar.copy(t[:csz, :], tps[:csz, :])
                PT_sb.append(t)
        aps = ps_tile([CLS, NIT])
        for k in range(8):
            ksz = V_CH[k % 4][1] - V_CH[k % 4][0]
            nc.tensor.matmul(aps[:, :], lhsT=wg_k[k][:ksz, :CLS],
                             rhs=PT_sb[k][:ksz, :], start=(k == 0), stop=False)
        nc.tensor.matmul(aps[:, :], lhsT=bg_row[:1, :CLS],
                         rhs=ones_bf[:1, :NIT], start=False, stop=True)
        out_sb = C.tile([CLS, NIT], F32)
        nc.scalar.copy(out_sb[:], aps[:, :])
        nc.sync.dma_start(out=out_d.ap(), in_=out_sb[:])


def _get_nc():
    global _CACHED_NC
    if _CACHED_NC is None:
        _CACHED_NC = _build_nc()
    return _CACHED_NC


def make_in_maps(inputs):
    x1 = np.asarray(inputs["x1"])
    x2 = np.asarray(inputs["x2"])
    f32 = lambda k: np.ascontiguousarray(np.asarray(inputs[k], dtype=np.float32))
    bf = lambda a: np.ascontiguousarray(np.asarray(a, dtype=np.float32)).astype(BF_NP)
    ii, jj = np.meshgrid(np.arange(L), np.arange(L), indexing="ij")
    dmask = (np.abs(ii - jj) >= 10).astype(np.float32)
    bdist = np.full((128, 1), np.asarray(inputs["b_dist"], np.float32).reshape(-1)[0],
                    np.float32)

    shared = {
        "emb": bf(inputs["emb"]),
        "wi": bf(inputs["Wi"]), "wp": bf(inputs["Wp"]), "wa": bf(inputs["Wa"]),
        "wc": bf(inputs["Wc"]), "wg": bf(inputs["Wg"]),
        "bi": f32("bi").reshape(-1, 1), "bp": f32("bp").reshape(-1, 1),
        "ba_row": bf(np.asarray(inputs["ba"]).reshape(1, -1)),
        "bc_row": bf(np.asarray(inputs["bc"]).reshape(1, -1)),
        "bg_row": bf(np.asarray(inputs["bg"]).reshape(1, -1)),
        "dmask": dmask.astype(BF_NP), "bdist": bdist,
    }
    in_maps = []
    for c in range(NCORES):
        sl = slice(c * NIT, (c + 1) * NIT)
        x1s = np.ascontiguousarray(x1[sl]).astype(np.int32)
        x2s = np.ascontiguousarray(x2[sl]).astype(np.int32)
        m = dict(shared)
        m["idx1"] = np.ascontiguousarray(x1s.reshape(-1).reshape(2 * NIT, 128).T)
        m["idx2"] = np.ascontiguousarray(x2s.reshape(-1).reshape(2 * NIT, 128).T)
        m["xi1"] = x1s
        m["xi2"] = x2s
        in_maps.append(m)
    return in_maps


def kernel(**inputs):
    nc = _get_nc()
    in_maps = make_in_maps(inputs)
    res = run_bass_kernel_spmd(nc, in_maps, core_ids=list(range(NCORES)))
    out = np.concatenate([r["out"].T for r in res.results], axis=0)
    return np.ascontiguousarray(out, dtype=np.float32)
